# revision 1
# baseline (speedup 1.0000x reference)
"""Trainium2 Bass kernel for nn_AwareDecoder segment first/last gather.

Problem: input [16, 2048, 1024] f32, number_mask [16, 2048] int64 with ids in
[0, 512]. For each segment id i in [0, 512): find first/last row-major token
position with that id, gather those rows of the flattened input, concat ->
out [512, 2048] f32.

Strategy (8 NeuronCores, segment-sharded - no collectives):
  core c owns segments [64c, 64c+64). Each core:
    - DMAs the (tiny, 256KB) id array, extracts int64 low words,
    - computes per-segment min/max token position with an fp16 eq/select/
      reduce sweep on the vector engine. Token chunks sit on partitions and
      positions are encoded chunk-LOCALLY (values <= 256, fp16-exact) so the
      four mult/reduce passes run in the DVE 2x packed mode; the global
      position is reconstructed in the tiny post-transpose stage,
    - PE-transpose + free-axis reduce for the cross-partition combine,
    - gathers its 64 first + 64 last rows (4KB each) straight from HBM with
      one hardware indirect DMA (reads only 512KB of the 128MB input),
    - writes its [64, 2048] slice of the output.
Host concatenates the 8 slices.
"""
import numpy as np

import concourse.bass as bass
import concourse.tile as tile
from concourse import bacc, mybir
from concourse import bass_utils
from concourse.masks import make_identity

P = 128            # partitions
L = 32768          # B*S tokens
H = 1024           # hidden
NSEG = 512         # segments
NCORES = 8
SEG_PER_CORE = NSEG // NCORES            # 64
TOK_PER_PART = L // P                    # 256 tokens per partition
F32 = mybir.dt.float32
F16 = mybir.dt.float16
I32 = mybir.dt.int32


def build_nc():
    nc = bacc.Bacc("TRN2", target_bir_lowering=False, debug=False)

    x = nc.dram_tensor("x", [L, H], F32, kind="ExternalInput")
    # number_mask int64 raw bytes as int32 (lo, hi) pairs; partition p covers
    # tokens [p*256, (p+1)*256).
    idpairs = nc.dram_tensor("idpairs", [P, TOK_PER_PART, 2], I32, kind="ExternalInput")
    # packed fp16 consts (per-core): [c8hi (8*256) | c8lo (8*256) | posmin | posmax]
    cpack_in = nc.dram_tensor("cpack", [P, 18 * TOK_PER_PART], F16,
                              kind="ExternalInput")
    # global-position bases for the post-transpose decode:
    # rows 0..63   (min side): base[s, p] = (127 - p) * 256
    # rows 64..127 (max side): base[s, p] = p * 256
    base_in = nc.dram_tensor("base", [2, SEG_PER_CORE, P], F32, kind="ExternalInput")
    out = nc.dram_tensor("out", [SEG_PER_CORE, 2 * H], F32, kind="ExternalOutput")

    with tile.TileContext(nc) as tc:
        with tc.tile_pool(name="sb", bufs=1) as sb, \
             tc.tile_pool(name="big", bufs=1) as big, \
             tc.tile_pool(name="ps", bufs=1, space="PSUM") as ps:

            # ---- load ids, extract low int32 words, cast to fp16 ----
            idp_t = sb.tile([P, TOK_PER_PART, 2], I32)
            nc.sync.dma_start(idp_t[:], idpairs.ap())
            cpack = sb.tile([P, 18 * TOK_PER_PART], F16)
            nc.scalar.dma_start(cpack[:], cpack_in.ap())
            c8hi_t = cpack[:, 0:8 * TOK_PER_PART].rearrange(
                "p (a t) -> p a t", a=8)
            c8lo_t = cpack[:, 8 * TOK_PER_PART:16 * TOK_PER_PART].rearrange(
                "p (a t) -> p a t", a=8)
            posmin = cpack[:, 16 * TOK_PER_PART:17 * TOK_PER_PART]
            posmax = cpack[:, 17 * TOK_PER_PART:18 * TOK_PER_PART]
            base_t = sb.tile([P, P], F32)
            nc.gpsimd.dma_start(base_t[:], base_in.ap().rearrange("a s p -> (a s) p"))

            # ---- factorized seg compare: id>>3 == base/8 + m, id&7 == lo ----
            hi_i = sb.tile([P, TOK_PER_PART], I32)
            nc.vector.tensor_scalar(hi_i[:], idp_t[:, :, 0], 3, None,
                                    op0=mybir.AluOpType.arith_shift_right)
            lo_i = sb.tile([P, TOK_PER_PART], I32)
            nc.vector.tensor_scalar(lo_i[:], idp_t[:, :, 0], 7, None,
                                    op0=mybir.AluOpType.bitwise_and)
            hi_f = sb.tile([P, TOK_PER_PART], F16)
            nc.vector.tensor_copy(hi_f[:], hi_i[:])
            lo_f = sb.tile([P, TOK_PER_PART], F16)
            nc.vector.tensor_copy(lo_f[:], lo_i[:])

            eq_hi = sb.tile([P, 8, TOK_PER_PART], F16)
            nc.vector.tensor_tensor(
                out=eq_hi[:],
                in0=hi_f[:].unsqueeze(1).broadcast_to([P, 8, TOK_PER_PART]),
                in1=c8hi_t, op=mybir.AluOpType.is_equal)
            eq_lo = sb.tile([P, 8, TOK_PER_PART], F16)
            nc.vector.tensor_tensor(
                out=eq_lo[:],
                in0=lo_f[:].unsqueeze(1).broadcast_to([P, 8, TOK_PER_PART]),
                in1=c8lo_t, op=mybir.AluOpType.is_equal)
            eqlo_min = sb.tile([P, 8, TOK_PER_PART], F16)
            nc.vector.tensor_tensor(
                out=eqlo_min[:], in0=eq_lo[:],
                in1=posmin.unsqueeze(1).broadcast_to([P, 8, TOK_PER_PART]),
                op=mybir.AluOpType.mult)
            eqlo_max = sb.tile([P, 8, TOK_PER_PART], F16)
            nc.vector.tensor_tensor(
                out=eqlo_max[:], in0=eq_lo[:],
                in1=posmax.unsqueeze(1).broadcast_to([P, 8, TOK_PER_PART]),
                op=mybir.AluOpType.mult)

            # ---- big fused candidate passes (2x) + reduces ----
            cand = big.tile([P, 8, 8, TOK_PER_PART], F16)
            nc.vector.tensor_tensor(
                out=cand[:],
                in0=eq_hi[:].unsqueeze(2).broadcast_to([P, 8, 8, TOK_PER_PART]),
                in1=eqlo_min[:].unsqueeze(1).broadcast_to([P, 8, 8, TOK_PER_PART]),
                op=mybir.AluOpType.mult)
            # TT-max tree (2x) then small reduce: 256 -> 32 -> 1
            red = sb.tile([P, P], F16)  # [:, :64] min-enc, [:, 64:] max-enc
            c3 = cand[:].rearrange("p a b t -> p (a b) t")
            lv1 = big.tile([P, SEG_PER_CORE, 128], F16, tag="lv1")
            nc.vector.tensor_tensor(out=lv1[:], in0=c3[:, :, 0:128],
                                    in1=c3[:, :, 128:256], op=mybir.AluOpType.max)
            lv2 = sb.tile([P, SEG_PER_CORE, 64], F16, tag="lv2")
            nc.vector.tensor_tensor(out=lv2[:], in0=lv1[:, :, 0:64],
                                    in1=lv1[:, :, 64:128], op=mybir.AluOpType.max)
            lv3 = sb.tile([P, SEG_PER_CORE, 32], F16, tag="lv3")
            nc.vector.tensor_tensor(out=lv3[:], in0=lv2[:, :, 0:32],
                                    in1=lv2[:, :, 32:64], op=mybir.AluOpType.max)
            nc.vector.tensor_reduce(red[:, 0:SEG_PER_CORE], lv3[:],
                                    axis=mybir.AxisListType.X,
                                    op=mybir.AluOpType.max)
            cand2 = big.tile([P, 8, 8, TOK_PER_PART], F16)
            nc.vector.tensor_tensor(
                out=cand2[:],
                in0=eq_hi[:].unsqueeze(2).broadcast_to([P, 8, 8, TOK_PER_PART]),
                in1=eqlo_max[:].unsqueeze(1).broadcast_to([P, 8, 8, TOK_PER_PART]),
                op=mybir.AluOpType.mult)
            c3b = cand2[:].rearrange("p a b t -> p (a b) t")
            lv1b = big.tile([P, SEG_PER_CORE, 128], F16, tag="lv1")
            nc.vector.tensor_tensor(out=lv1b[:], in0=c3b[:, :, 0:128],
                                    in1=c3b[:, :, 128:256], op=mybir.AluOpType.max)
            lv2b = sb.tile([P, SEG_PER_CORE, 64], F16, tag="lv2")
            nc.vector.tensor_tensor(out=lv2b[:], in0=lv1b[:, :, 0:64],
                                    in1=lv1b[:, :, 64:128], op=mybir.AluOpType.max)
            lv3b = sb.tile([P, SEG_PER_CORE, 32], F16, tag="lv3")
            nc.vector.tensor_tensor(out=lv3b[:], in0=lv2b[:, :, 0:32],
                                    in1=lv2b[:, :, 32:64], op=mybir.AluOpType.max)
            nc.vector.tensor_reduce(red[:, SEG_PER_CORE:P], lv3b[:],
                                    axis=mybir.AxisListType.X,
                                    op=mybir.AluOpType.max)

            # ---- cross-partition combine, decode, gather ----
            ident = sb.tile([P, P], F16)
            make_identity(nc, ident[:])
            red_t = ps.tile([P, P], F16)
            nc.tensor.transpose(out=red_t[:], in_=red[:], identity=ident[:])
            mask = sb.tile([P, P], F32)
            nc.vector.tensor_scalar(mask[:], red_t[:], 0.0, None,
                                    op0=mybir.AluOpType.is_gt)
            glob = sb.tile([P, P], F32)
            nc.vector.tensor_tensor(out=glob[:], in0=red_t[:], in1=base_t[:],
                                    op=mybir.AluOpType.add)
            nc.vector.tensor_tensor(out=glob[:], in0=glob[:], in1=mask[:],
                                    op=mybir.AluOpType.mult)
            enc = sb.tile([P, 1], F32)
            nc.vector.tensor_reduce(enc[:], glob[:],
                                    axis=mybir.AxisListType.X,
                                    op=mybir.AluOpType.max)
            idx_f = sb.tile([P, 1], F32)
            nc.vector.tensor_scalar(idx_f[0:SEG_PER_CORE, :], enc[0:SEG_PER_CORE, :],
                                    -1.0, float(L),
                                    op0=mybir.AluOpType.mult,
                                    op1=mybir.AluOpType.add)
            nc.vector.tensor_scalar_add(idx_f[SEG_PER_CORE:P, :],
                                        enc[SEG_PER_CORE:P, :], -1.0)
            idx_i = sb.tile([P, 1], I32)
            nc.vector.tensor_copy(idx_i[:], idx_f[:])
            rows = big.tile([P, H], F32)
            nc.gpsimd.indirect_dma_start(
                out=rows[:], out_offset=None, in_=x.ap(),
                in_offset=bass.IndirectOffsetOnAxis(ap=idx_i[:, 0:1], axis=0))
            nc.gpsimd.dma_start(out.ap()[:, 0:H], rows[0:SEG_PER_CORE, :])
            nc.sync.dma_start(out.ap()[:, H:2 * H], rows[SEG_PER_CORE:P, :])

    nc.compile()
    return nc


_NC = None


def _get_nc():
    global _NC
    if _NC is None:
        _NC = build_nc()
    return _NC


def make_in_maps(input, number_mask):
    x = np.ascontiguousarray(np.asarray(input), dtype=np.float32).reshape(L, H)
    nm = np.ascontiguousarray(np.asarray(number_mask))
    if nm.dtype != np.int64:
        nm = nm.astype(np.int64)
    idpairs = nm.reshape(L).view(np.int32).reshape(P, TOK_PER_PART, 2)
    c8lo = np.repeat(np.arange(8, dtype=np.float16), TOK_PER_PART)
    f = np.arange(TOK_PER_PART, dtype=np.float16)
    pcol = np.arange(P, dtype=np.float32)
    base = np.empty((2, SEG_PER_CORE, P), dtype=np.float32)
    base[0] = (P - 1 - pcol) * TOK_PER_PART
    base[1] = pcol * TOK_PER_PART
    in_maps = []
    for c in range(NCORES):
        c8hi = np.repeat(np.arange(8, dtype=np.float16) + c * 8, TOK_PER_PART)
        cpack = np.tile(np.concatenate([c8hi, c8lo, TOK_PER_PART - f, f + 1]),
                        (P, 1))
        in_maps.append({"x": x, "idpairs": idpairs, "cpack": cpack,
                        "base": base})
    return in_maps


def kernel(input, number_mask, n, concat, **_):
    assert int(n) == NSEG and int(concat) == 1
    nc = _get_nc()
    in_maps = make_in_maps(input, number_mask)
    res = bass_utils.run_bass_kernel_spmd(nc, in_maps, core_ids=list(range(NCORES)))
    return np.concatenate([res.results[c]["out"] for c in range(NCORES)], axis=0)



# revision 6
# speedup vs baseline: 2.2756x; 2.2756x over previous
"""Trainium2 Bass kernel for nn_AwareDecoder segment first/last gather.

Problem: input [16, 2048, 1024] f32, number_mask [16, 2048] int64 with ids in
[0, 512]. For each segment id i in [0, 512): find first/last row-major token
position with that id, gather those rows of the flattened input, concat ->
out [512, 2048] f32.

Fast path (8 NeuronCores, segment-sharded - no collectives):
  core c owns segments [64c, 64c+64). Token t = (p, f) with partition
  p = t >> 8 and in-row offset f = t & 255. Within one 256-token row every
  occurring id appears at most once (host-verified; true for the reference's
  arange % 513 mask since 256 < 513), so a single GPSIMD local_scatter builds
  the whole per-row segment table in one shot:

    idx16[p, f] = id[p, f] - 64c   (negative / out-of-range ids are ignored
                                    or land in unused table slots)
    tab[p, v]   = f + 1 where id[p, f] == 64c + v, else 0

  The cross-row combine reuses the encode/transpose/decode trick: cols 0:64
  of a [128, 128] tile hold -tab (first side), cols 64:128 hold +tab (last
  side); PE-transpose, add per-side bases ((127-p)*256 + 257 resp. 256p - 1),
  mask zeros, free-axis max-reduce, then one hardware indirect DMA gathers
  the 64+64 rows (512KB of the 128MB input) and two DMAs write the
  [64, 2048] output slice. Host concatenates the 8 slices.

Fallback (any per-row duplicate id): the original eq/select/reduce sweep
kernel, compiled lazily.
"""
import numpy as np

import concourse.bass as bass
import concourse.tile as tile
from concourse import bacc, mybir
from concourse import bass_utils

P = 128            # partitions
L = 32768          # B*S tokens
H = 1024           # hidden
NSEG = 512         # segments
NCORES = 8
SEG_PER_CORE = NSEG // NCORES            # 64
TOK_PER_PART = L // P                    # 256 tokens per partition
NELEM = 514        # local_scatter table width (>= 513, even)
F32 = mybir.dt.float32
F16 = mybir.dt.float16
I32 = mybir.dt.int32
I16 = mybir.dt.int16


def build_nc():
    nc = bacc.Bacc("TRN2", target_bir_lowering=False, debug=False)

    x = nc.dram_tensor("x", [L, H], F32, kind="ExternalInput")
    # number_mask int64 raw bytes as int32 (lo, hi) pairs; partition p covers
    # tokens [p*256, (p+1)*256).
    idpairs = nc.dram_tensor("idpairs", [P, TOK_PER_PART, 2], I32, kind="ExternalInput")
    # per-core segment base (64*c), replicated over partitions
    segb_in = nc.dram_tensor("segb", [P, 1], F32, kind="ExternalInput")
    # fp16 consts: [identity (128) | enc = f+1 (256)]
    cpk_in = nc.dram_tensor("cpk", [P, P + TOK_PER_PART], F16, kind="ExternalInput")
    # decode bases: rows 0..63  (min side): (127 - p) * 256 + 257
    #               rows 64..127 (max side): 256 * p - 1
    base_in = nc.dram_tensor("base", [P, P], F32, kind="ExternalInput")
    out = nc.dram_tensor("out", [SEG_PER_CORE, 2 * H], F32, kind="ExternalOutput")

    with tile.TileContext(nc) as tc:
        with tc.tile_pool(name="sb", bufs=1) as sb, \
             tc.tile_pool(name="ps", bufs=1, space="PSUM") as ps:

            segb_t = sb.tile([P, 1], F32)
            nc.scalar.dma_start(segb_t[:], segb_in.ap())
            idp_t = sb.tile([P, TOK_PER_PART, 2], I32)
            nc.sync.dma_start(idp_t[:], idpairs.ap())
            cpk_t = sb.tile([P, P + TOK_PER_PART], F16)
            nc.scalar.dma_start(cpk_t[:], cpk_in.ap())
            base_t = sb.tile([P, P], F32)
            nc.gpsimd.dma_start(base_t[:], base_in.ap())
            ident = cpk_t[:, 0:P]
            enc = cpk_t[:, P:P + TOK_PER_PART]

            # rebase ids to the core's segment range and narrow to int16
            idx16 = sb.tile([P, TOK_PER_PART], I16)
            nc.vector.tensor_scalar(idx16[:], idp_t[:, :, 0], segb_t[:, 0:1], None,
                                    op0=mybir.AluOpType.subtract)

            # one scatter builds the whole per-row segment table
            tab = sb.tile([P, NELEM], F16)
            nc.gpsimd.local_scatter(tab[:], enc, idx16[:],
                                    channels=P, num_elems=NELEM,
                                    num_idxs=TOK_PER_PART)

            # [:, 0:64] = -tab (first side), [:, 64:128] = +tab (last side)
            red = sb.tile([P, P], F16)
            nc.vector.tensor_scalar(red[:, 0:SEG_PER_CORE], tab[:, 0:SEG_PER_CORE],
                                    -1.0, None, op0=mybir.AluOpType.mult)
            nc.vector.tensor_copy(red[:, SEG_PER_CORE:P], tab[:, 0:SEG_PER_CORE])

            # cross-partition combine: transpose, add bases, mask, max-reduce
            red_t = ps.tile([P, P], F16)
            nc.tensor.transpose(out=red_t[:], in_=red[:], identity=ident)
            summ = sb.tile([P, P], F32)
            nc.vector.tensor_tensor(out=summ[:], in0=red_t[:], in1=base_t[:],
                                    op=mybir.AluOpType.add)
            glob = sb.tile([P, P], F32)
            nc.vector.scalar_tensor_tensor(out=glob[:], in0=red_t[:], scalar=0.0,
                                           in1=summ[:],
                                           op0=mybir.AluOpType.not_equal,
                                           op1=mybir.AluOpType.mult)
            enc1 = sb.tile([P, 1], F32)
            nc.vector.tensor_reduce(enc1[:], glob[:],
                                    axis=mybir.AxisListType.X,
                                    op=mybir.AluOpType.max)
            # decode: first = L - enc, last = enc
            idx_f = sb.tile([P, 1], F32)
            nc.vector.tensor_scalar(idx_f[0:SEG_PER_CORE, :], enc1[0:SEG_PER_CORE, :],
                                    -1.0, float(L),
                                    op0=mybir.AluOpType.mult,
                                    op1=mybir.AluOpType.add)
            nc.vector.tensor_copy(idx_f[SEG_PER_CORE:P, :], enc1[SEG_PER_CORE:P, :])
            idx_i = sb.tile([P, 1], I32)
            nc.vector.tensor_copy(idx_i[:], idx_f[:])

            rows = sb.tile([P, H], F32)
            nc.gpsimd.indirect_dma_start(
                out=rows[:], out_offset=None, in_=x.ap(),
                in_offset=bass.IndirectOffsetOnAxis(ap=idx_i[:, 0:1], axis=0))
            nc.scalar.dma_start(out.ap()[:, 0:H], rows[0:SEG_PER_CORE, :])
            nc.sync.dma_start(out.ap()[:, H:2 * H], rows[SEG_PER_CORE:P, :])

    nc.compile()
    return nc


def make_in_maps(input, number_mask):
    x = np.ascontiguousarray(np.asarray(input), dtype=np.float32).reshape(L, H)
    nm = np.ascontiguousarray(np.asarray(number_mask))
    if nm.dtype != np.int64:
        nm = nm.astype(np.int64)
    idpairs = nm.reshape(L).view(np.int32).reshape(P, TOK_PER_PART, 2)
    ident = np.eye(P, dtype=np.float16)
    enc = np.tile(np.arange(1, TOK_PER_PART + 1, dtype=np.float16), (P, 1))
    cpk = np.concatenate([ident, enc], axis=1)
    pcol = np.arange(P, dtype=np.float32)
    base = np.empty((P, P), dtype=np.float32)
    base[0:SEG_PER_CORE] = (P - 1 - pcol) * TOK_PER_PART + TOK_PER_PART + 1
    base[SEG_PER_CORE:P] = pcol * TOK_PER_PART - 1
    in_maps = []
    for c in range(NCORES):
        segb = np.full((P, 1), c * SEG_PER_CORE, dtype=np.float32)
        in_maps.append({"x": x, "idpairs": idpairs, "segb": segb,
                        "cpk": cpk, "base": base})
    return in_maps


# ---------------------------------------------------------------------------
# Fallback: original eq/select/reduce sweep (handles per-row duplicate ids).
# ---------------------------------------------------------------------------

def build_nc_general():
    from concourse.masks import make_identity

    nc = bacc.Bacc("TRN2", target_bir_lowering=False, debug=False)

    x = nc.dram_tensor("x", [L, H], F32, kind="ExternalInput")
    idpairs = nc.dram_tensor("idpairs", [P, TOK_PER_PART, 2], I32, kind="ExternalInput")
    cpack_in = nc.dram_tensor("cpack", [P, 18 * TOK_PER_PART], F16,
                              kind="ExternalInput")
    base_in = nc.dram_tensor("base", [2, SEG_PER_CORE, P], F32, kind="ExternalInput")
    out = nc.dram_tensor("out", [SEG_PER_CORE, 2 * H], F32, kind="ExternalOutput")

    with tile.TileContext(nc) as tc:
        with tc.tile_pool(name="sb", bufs=1) as sb, \
             tc.tile_pool(name="big", bufs=1) as big, \
             tc.tile_pool(name="ps", bufs=1, space="PSUM") as ps:

            idp_t = sb.tile([P, TOK_PER_PART, 2], I32)
            nc.sync.dma_start(idp_t[:], idpairs.ap())
            cpack = sb.tile([P, 18 * TOK_PER_PART], F16)
            nc.scalar.dma_start(cpack[:], cpack_in.ap())
            c8hi_t = cpack[:, 0:8 * TOK_PER_PART].rearrange(
                "p (a t) -> p a t", a=8)
            c8lo_t = cpack[:, 8 * TOK_PER_PART:16 * TOK_PER_PART].rearrange(
                "p (a t) -> p a t", a=8)
            posmin = cpack[:, 16 * TOK_PER_PART:17 * TOK_PER_PART]
            posmax = cpack[:, 17 * TOK_PER_PART:18 * TOK_PER_PART]
            base_t = sb.tile([P, P], F32)
            nc.gpsimd.dma_start(base_t[:], base_in.ap().rearrange("a s p -> (a s) p"))

            hi_i = sb.tile([P, TOK_PER_PART], I32)
            nc.vector.tensor_scalar(hi_i[:], idp_t[:, :, 0], 3, None,
                                    op0=mybir.AluOpType.arith_shift_right)
            lo_i = sb.tile([P, TOK_PER_PART], I32)
            nc.vector.tensor_scalar(lo_i[:], idp_t[:, :, 0], 7, None,
                                    op0=mybir.AluOpType.bitwise_and)
            hi_f = sb.tile([P, TOK_PER_PART], F16)
            nc.vector.tensor_copy(hi_f[:], hi_i[:])
            lo_f = sb.tile([P, TOK_PER_PART], F16)
            nc.vector.tensor_copy(lo_f[:], lo_i[:])

            eq_hi = sb.tile([P, 8, TOK_PER_PART], F16)
            nc.vector.tensor_tensor(
                out=eq_hi[:],
                in0=hi_f[:].unsqueeze(1).broadcast_to([P, 8, TOK_PER_PART]),
                in1=c8hi_t, op=mybir.AluOpType.is_equal)
            eq_lo = sb.tile([P, 8, TOK_PER_PART], F16)
            nc.vector.tensor_tensor(
                out=eq_lo[:],
                in0=lo_f[:].unsqueeze(1).broadcast_to([P, 8, TOK_PER_PART]),
                in1=c8lo_t, op=mybir.AluOpType.is_equal)
            eqlo_min = sb.tile([P, 8, TOK_PER_PART], F16)
            nc.vector.tensor_tensor(
                out=eqlo_min[:], in0=eq_lo[:],
                in1=posmin.unsqueeze(1).broadcast_to([P, 8, TOK_PER_PART]),
                op=mybir.AluOpType.mult)
            eqlo_max = sb.tile([P, 8, TOK_PER_PART], F16)
            nc.vector.tensor_tensor(
                out=eqlo_max[:], in0=eq_lo[:],
                in1=posmax.unsqueeze(1).broadcast_to([P, 8, TOK_PER_PART]),
                op=mybir.AluOpType.mult)

            cand = big.tile([P, 8, 8, TOK_PER_PART], F16)
            nc.vector.tensor_tensor(
                out=cand[:],
                in0=eq_hi[:].unsqueeze(2).broadcast_to([P, 8, 8, TOK_PER_PART]),
                in1=eqlo_min[:].unsqueeze(1).broadcast_to([P, 8, 8, TOK_PER_PART]),
                op=mybir.AluOpType.mult)
            red = sb.tile([P, P], F16)
            c3 = cand[:].rearrange("p a b t -> p (a b) t")
            lv1 = big.tile([P, SEG_PER_CORE, 128], F16, tag="lv1")
            nc.vector.tensor_tensor(out=lv1[:], in0=c3[:, :, 0:128],
                                    in1=c3[:, :, 128:256], op=mybir.AluOpType.max)
            lv2 = sb.tile([P, SEG_PER_CORE, 64], F16, tag="lv2")
            nc.vector.tensor_tensor(out=lv2[:], in0=lv1[:, :, 0:64],
                                    in1=lv1[:, :, 64:128], op=mybir.AluOpType.max)
            lv3 = sb.tile([P, SEG_PER_CORE, 32], F16, tag="lv3")
            nc.vector.tensor_tensor(out=lv3[:], in0=lv2[:, :, 0:32],
                                    in1=lv2[:, :, 32:64], op=mybir.AluOpType.max)
            nc.vector.tensor_reduce(red[:, 0:SEG_PER_CORE], lv3[:],
                                    axis=mybir.AxisListType.X,
                                    op=mybir.AluOpType.max)
            cand2 = big.tile([P, 8, 8, TOK_PER_PART], F16)
            nc.vector.tensor_tensor(
                out=cand2[:],
                in0=eq_hi[:].unsqueeze(2).broadcast_to([P, 8, 8, TOK_PER_PART]),
                in1=eqlo_max[:].unsqueeze(1).broadcast_to([P, 8, 8, TOK_PER_PART]),
                op=mybir.AluOpType.mult)
            c3b = cand2[:].rearrange("p a b t -> p (a b) t")
            lv1b = big.tile([P, SEG_PER_CORE, 128], F16, tag="lv1")
            nc.vector.tensor_tensor(out=lv1b[:], in0=c3b[:, :, 0:128],
                                    in1=c3b[:, :, 128:256], op=mybir.AluOpType.max)
            lv2b = sb.tile([P, SEG_PER_CORE, 64], F16, tag="lv2")
            nc.vector.tensor_tensor(out=lv2b[:], in0=lv1b[:, :, 0:64],
                                    in1=lv1b[:, :, 64:128], op=mybir.AluOpType.max)
            lv3b = sb.tile([P, SEG_PER_CORE, 32], F16, tag="lv3")
            nc.vector.tensor_tensor(out=lv3b[:], in0=lv2b[:, :, 0:32],
                                    in1=lv2b[:, :, 32:64], op=mybir.AluOpType.max)
            nc.vector.tensor_reduce(red[:, SEG_PER_CORE:P], lv3b[:],
                                    axis=mybir.AxisListType.X,
                                    op=mybir.AluOpType.max)

            ident = sb.tile([P, P], F16)
            make_identity(nc, ident[:])
            red_t = ps.tile([P, P], F16)
            nc.tensor.transpose(out=red_t[:], in_=red[:], identity=ident[:])
            mask = sb.tile([P, P], F32)
            nc.vector.tensor_scalar(mask[:], red_t[:], 0.0, None,
                                    op0=mybir.AluOpType.is_gt)
            glob = sb.tile([P, P], F32)
            nc.vector.tensor_tensor(out=glob[:], in0=red_t[:], in1=base_t[:],
                                    op=mybir.AluOpType.add)
            nc.vector.tensor_tensor(out=glob[:], in0=glob[:], in1=mask[:],
                                    op=mybir.AluOpType.mult)
            enc = sb.tile([P, 1], F32)
            nc.vector.tensor_reduce(enc[:], glob[:],
                                    axis=mybir.AxisListType.X,
                                    op=mybir.AluOpType.max)
            idx_f = sb.tile([P, 1], F32)
            nc.vector.tensor_scalar(idx_f[0:SEG_PER_CORE, :], enc[0:SEG_PER_CORE, :],
                                    -1.0, float(L),
                                    op0=mybir.AluOpType.mult,
                                    op1=mybir.AluOpType.add)
            nc.vector.tensor_scalar_add(idx_f[SEG_PER_CORE:P, :],
                                        enc[SEG_PER_CORE:P, :], -1.0)
            idx_i = sb.tile([P, 1], I32)
            nc.vector.tensor_copy(idx_i[:], idx_f[:])
            rows = big.tile([P, H], F32)
            nc.gpsimd.indirect_dma_start(
                out=rows[:], out_offset=None, in_=x.ap(),
                in_offset=bass.IndirectOffsetOnAxis(ap=idx_i[:, 0:1], axis=0))
            nc.gpsimd.dma_start(out.ap()[:, 0:H], rows[0:SEG_PER_CORE, :])
            nc.sync.dma_start(out.ap()[:, H:2 * H], rows[SEG_PER_CORE:P, :])

    nc.compile()
    return nc


def make_in_maps_general(input, number_mask):
    x = np.ascontiguousarray(np.asarray(input), dtype=np.float32).reshape(L, H)
    nm = np.ascontiguousarray(np.asarray(number_mask))
    if nm.dtype != np.int64:
        nm = nm.astype(np.int64)
    idpairs = nm.reshape(L).view(np.int32).reshape(P, TOK_PER_PART, 2)
    c8lo = np.repeat(np.arange(8, dtype=np.float16), TOK_PER_PART)
    f = np.arange(TOK_PER_PART, dtype=np.float16)
    pcol = np.arange(P, dtype=np.float32)
    base = np.empty((2, SEG_PER_CORE, P), dtype=np.float32)
    base[0] = (P - 1 - pcol) * TOK_PER_PART
    base[1] = pcol * TOK_PER_PART
    in_maps = []
    for c in range(NCORES):
        c8hi = np.repeat(np.arange(8, dtype=np.float16) + c * 8, TOK_PER_PART)
        cpack = np.tile(np.concatenate([c8hi, c8lo, TOK_PER_PART - f, f + 1]),
                        (P, 1))
        in_maps.append({"x": x, "idpairs": idpairs, "cpack": cpack,
                        "base": base})
    return in_maps


_NC = None
_NC_GENERAL = None


def _get_nc():
    global _NC
    if _NC is None:
        _NC = build_nc()
    return _NC


def _get_nc_general():
    global _NC_GENERAL
    if _NC_GENERAL is None:
        _NC_GENERAL = build_nc_general()
    return _NC_GENERAL


def _rows_distinct(number_mask):
    ids = np.asarray(number_mask).reshape(P, TOK_PER_PART)
    s = np.sort(ids, axis=1)
    return not np.any(s[:, 1:] == s[:, :-1])


def kernel(input, number_mask, n, concat, **_):
    assert int(n) == NSEG and int(concat) == 1
    if _rows_distinct(number_mask):
        nc = _get_nc()
        in_maps = make_in_maps(input, number_mask)
    else:
        nc = _get_nc_general()
        in_maps = make_in_maps_general(input, number_mask)
    res = bass_utils.run_bass_kernel_spmd(nc, in_maps, core_ids=list(range(NCORES)))
    return np.concatenate([res.results[c]["out"] for c in range(NCORES)], axis=0)


# revision 11
# speedup vs baseline: 2.4960x; 1.0968x over previous
"""Trainium2 Bass kernel for nn_AwareDecoder segment first/last gather.

Problem: input [16, 2048, 1024] f32, number_mask [16, 2048] int64 with ids in
[0, 512]. For each segment id i in [0, 512): find first/last row-major token
position with that id, gather those rows of the flattened input, concat ->
out [512, 2048] f32.

Fast path (8 NeuronCores, segment-sharded - no collectives):
  core c owns segments [64c, 64c+64). Token t = (p, f) with partition
  p = t >> 8 and in-row offset f = t & 255. Within one 256-token row every
  occurring id appears at most once (host-verified; true for the reference's
  arange % 513 mask since 256 < 513), so a single GPSIMD local_scatter builds
  the whole per-row segment table in one shot:

    idx16[p, f] = id[p, f] - 64c   (negative / out-of-range ids are ignored
                                    or land in unused table slots)
    tab[p, v]   = f + 1 where id[p, f] == 64c + v, else 0

  The cross-row combine reuses the encode/transpose/decode trick: cols 0:64
  of a [128, 128] tile hold -tab (first side), cols 64:128 hold +tab (last
  side); PE-transpose, add per-side bases ((127-p)*256 + 257 resp. 256p - 1),
  mask zeros, free-axis max-reduce, then one hardware indirect DMA gathers
  the 64+64 rows (512KB of the 128MB input) and two DMAs write the
  [64, 2048] output slice. Host concatenates the 8 slices.

Fallback (any per-row duplicate id): the original eq/select/reduce sweep
kernel, compiled lazily.
"""
import numpy as np

import concourse.bass as bass
import concourse.tile as tile
from concourse import bacc, library_config, mybir
from concourse import bass_utils

P = 128            # partitions
L = 32768          # B*S tokens
H = 1024           # hidden
NSEG = 512         # segments
NCORES = 8
SEG_PER_CORE = NSEG // NCORES            # 64
TOK_PER_PART = L // P                    # 256 tokens per partition
NELEM = 514        # local_scatter table width (>= 513, even)
F32 = mybir.dt.float32
F16 = mybir.dt.float16
I32 = mybir.dt.int32
I16 = mybir.dt.int16


def build_nc():
    nc = bacc.Bacc("TRN2", target_bir_lowering=False, debug=False)

    x = nc.dram_tensor("x", [L, H], F32, kind="ExternalInput")
    # number_mask int64 raw bytes as int32 (lo, hi) pairs; partition p covers
    # tokens [p*256, (p+1)*256).
    idpairs = nc.dram_tensor("idpairs", [P, TOK_PER_PART, 2], I32, kind="ExternalInput")
    # per-core segment base (64*c), replicated over partitions
    segb_in = nc.dram_tensor("segb", [P, 1], F32, kind="ExternalInput")
    # fp16 consts: [identity (128) | enc = f+1 (256)]
    cpk_in = nc.dram_tensor("cpk", [P, P + TOK_PER_PART], F16, kind="ExternalInput")
    # decode bases: rows 0..63  (min side): (127 - p) * 256 + 257
    #               rows 64..127 (max side): 256 * p - 1
    base_in = nc.dram_tensor("base", [P, P], F32, kind="ExternalInput")
    out = nc.dram_tensor("out", [SEG_PER_CORE, 2 * H], F32, kind="ExternalOutput")

    with tile.TileContext(nc) as tc:
        with tc.tile_pool(name="sb", bufs=1) as sb, \
             tc.tile_pool(name="ps", bufs=1, space="PSUM") as ps:

            segb_t = sb.tile([P, 1], F32)
            nc.scalar.dma_start(segb_t[:], segb_in.ap())
            idp_t = sb.tile([P, TOK_PER_PART, 2], I32)
            nc.sync.dma_start(idp_t[0:P // 2], idpairs.ap()[0:P // 2])
            nc.scalar.dma_start(idp_t[P // 2:P], idpairs.ap()[P // 2:P])
            cpk_t = sb.tile([P, P + TOK_PER_PART], F16)
            nc.scalar.dma_start(cpk_t[:], cpk_in.ap())
            base_t = sb.tile([P, P], F32)
            nc.sync.dma_start(base_t[:], base_in.ap())
            ident = cpk_t[:, 0:P]
            enc = cpk_t[:, P:P + TOK_PER_PART]

            # rebase ids to the core's segment range and narrow to int16
            idx16 = sb.tile([P, TOK_PER_PART], I16)
            nc.vector.tensor_scalar(idx16[:], idp_t[:, :, 0], segb_t[:, 0:1], None,
                                    op0=mybir.AluOpType.subtract)

            # one scatter builds the whole per-row segment table
            tab = sb.tile([P, NELEM], F16)
            nc.gpsimd.local_scatter(tab[:], enc, idx16[:],
                                    channels=P, num_elems=NELEM,
                                    num_idxs=TOK_PER_PART)

            # [:, 0:64] = -tab (first side), [:, 64:128] = +tab (last side)
            red = sb.tile([P, P], F16)
            nc.vector.tensor_scalar(red[:, 0:SEG_PER_CORE], tab[:, 0:SEG_PER_CORE],
                                    -1.0, None, op0=mybir.AluOpType.mult)
            nc.vector.tensor_copy(red[:, SEG_PER_CORE:P], tab[:, 0:SEG_PER_CORE])

            # cross-partition combine: transpose, add bases, mask, max-reduce
            red_t = ps.tile([P, P], F16)
            nc.tensor.transpose(out=red_t[:], in_=red[:], identity=ident)
            summ = sb.tile([P, P], F32)
            nc.vector.tensor_tensor(out=summ[:], in0=red_t[:], in1=base_t[:],
                                    op=mybir.AluOpType.add)
            glob = sb.tile([P, P], F32)
            nc.vector.scalar_tensor_tensor(out=glob[:], in0=red_t[:], scalar=0.0,
                                           in1=summ[:],
                                           op0=mybir.AluOpType.not_equal,
                                           op1=mybir.AluOpType.mult)
            enc1 = sb.tile([P, 1], F32)
            nc.vector.tensor_reduce(enc1[:], glob[:],
                                    axis=mybir.AxisListType.X,
                                    op=mybir.AluOpType.max)
            # decode: first = L - enc, last = enc
            idx_f = sb.tile([P, 1], F32)
            nc.vector.tensor_scalar(idx_f[0:SEG_PER_CORE, :], enc1[0:SEG_PER_CORE, :],
                                    -1.0, float(L),
                                    op0=mybir.AluOpType.mult,
                                    op1=mybir.AluOpType.add)
            nc.vector.tensor_copy(idx_f[SEG_PER_CORE:P, :], enc1[SEG_PER_CORE:P, :])
            idx_i = sb.tile([P, 1], I32)
            nc.vector.tensor_copy(idx_i[:], idx_f[:])

            rows = sb.tile([P, H], F32)
            nc.gpsimd.indirect_dma_start(
                out=rows[:], out_offset=None, in_=x.ap(),
                in_offset=bass.IndirectOffsetOnAxis(ap=idx_i[:, 0:1], axis=0))
            nc.sync.dma_start(out.ap()[:, 0:H], rows[0:SEG_PER_CORE, :])
            nc.scalar.dma_start(out.ap()[:, H:2 * H], rows[SEG_PER_CORE:P, :])

    nc.compile()
    return nc


def make_in_maps(input, number_mask):
    x = np.ascontiguousarray(np.asarray(input), dtype=np.float32).reshape(L, H)
    nm = np.ascontiguousarray(np.asarray(number_mask))
    if nm.dtype != np.int64:
        nm = nm.astype(np.int64)
    idpairs = nm.reshape(L).view(np.int32).reshape(P, TOK_PER_PART, 2)
    ident = np.eye(P, dtype=np.float16)
    enc = np.tile(np.arange(1, TOK_PER_PART + 1, dtype=np.float16), (P, 1))
    cpk = np.concatenate([ident, enc], axis=1)
    pcol = np.arange(P, dtype=np.float32)
    base = np.empty((P, P), dtype=np.float32)
    base[0:SEG_PER_CORE] = (P - 1 - pcol) * TOK_PER_PART + TOK_PER_PART + 1
    base[SEG_PER_CORE:P] = pcol * TOK_PER_PART - 1
    in_maps = []
    for c in range(NCORES):
        segb = np.full((P, 1), c * SEG_PER_CORE, dtype=np.float32)
        in_maps.append({"x": x, "idpairs": idpairs, "segb": segb,
                        "cpk": cpk, "base": base})
    return in_maps


# ---------------------------------------------------------------------------
# Fallback: original eq/select/reduce sweep (handles per-row duplicate ids).
# ---------------------------------------------------------------------------

def build_nc_general():
    from concourse.masks import make_identity

    nc = bacc.Bacc("TRN2", target_bir_lowering=False, debug=False)

    x = nc.dram_tensor("x", [L, H], F32, kind="ExternalInput")
    idpairs = nc.dram_tensor("idpairs", [P, TOK_PER_PART, 2], I32, kind="ExternalInput")
    cpack_in = nc.dram_tensor("cpack", [P, 18 * TOK_PER_PART], F16,
                              kind="ExternalInput")
    base_in = nc.dram_tensor("base", [2, SEG_PER_CORE, P], F32, kind="ExternalInput")
    out = nc.dram_tensor("out", [SEG_PER_CORE, 2 * H], F32, kind="ExternalOutput")

    with tile.TileContext(nc) as tc:
        with tc.tile_pool(name="sb", bufs=1) as sb, \
             tc.tile_pool(name="big", bufs=1) as big, \
             tc.tile_pool(name="ps", bufs=1, space="PSUM") as ps:

            idp_t = sb.tile([P, TOK_PER_PART, 2], I32)
            nc.sync.dma_start(idp_t[:], idpairs.ap())
            cpack = sb.tile([P, 18 * TOK_PER_PART], F16)
            nc.scalar.dma_start(cpack[:], cpack_in.ap())
            c8hi_t = cpack[:, 0:8 * TOK_PER_PART].rearrange(
                "p (a t) -> p a t", a=8)
            c8lo_t = cpack[:, 8 * TOK_PER_PART:16 * TOK_PER_PART].rearrange(
                "p (a t) -> p a t", a=8)
            posmin = cpack[:, 16 * TOK_PER_PART:17 * TOK_PER_PART]
            posmax = cpack[:, 17 * TOK_PER_PART:18 * TOK_PER_PART]
            base_t = sb.tile([P, P], F32)
            nc.gpsimd.dma_start(base_t[:], base_in.ap().rearrange("a s p -> (a s) p"))

            hi_i = sb.tile([P, TOK_PER_PART], I32)
            nc.vector.tensor_scalar(hi_i[:], idp_t[:, :, 0], 3, None,
                                    op0=mybir.AluOpType.arith_shift_right)
            lo_i = sb.tile([P, TOK_PER_PART], I32)
            nc.vector.tensor_scalar(lo_i[:], idp_t[:, :, 0], 7, None,
                                    op0=mybir.AluOpType.bitwise_and)
            hi_f = sb.tile([P, TOK_PER_PART], F16)
            nc.vector.tensor_copy(hi_f[:], hi_i[:])
            lo_f = sb.tile([P, TOK_PER_PART], F16)
            nc.vector.tensor_copy(lo_f[:], lo_i[:])

            eq_hi = sb.tile([P, 8, TOK_PER_PART], F16)
            nc.vector.tensor_tensor(
                out=eq_hi[:],
                in0=hi_f[:].unsqueeze(1).broadcast_to([P, 8, TOK_PER_PART]),
                in1=c8hi_t, op=mybir.AluOpType.is_equal)
            eq_lo = sb.tile([P, 8, TOK_PER_PART], F16)
            nc.vector.tensor_tensor(
                out=eq_lo[:],
                in0=lo_f[:].unsqueeze(1).broadcast_to([P, 8, TOK_PER_PART]),
                in1=c8lo_t, op=mybir.AluOpType.is_equal)
            eqlo_min = sb.tile([P, 8, TOK_PER_PART], F16)
            nc.vector.tensor_tensor(
                out=eqlo_min[:], in0=eq_lo[:],
                in1=posmin.unsqueeze(1).broadcast_to([P, 8, TOK_PER_PART]),
                op=mybir.AluOpType.mult)
            eqlo_max = sb.tile([P, 8, TOK_PER_PART], F16)
            nc.vector.tensor_tensor(
                out=eqlo_max[:], in0=eq_lo[:],
                in1=posmax.unsqueeze(1).broadcast_to([P, 8, TOK_PER_PART]),
                op=mybir.AluOpType.mult)

            cand = big.tile([P, 8, 8, TOK_PER_PART], F16)
            nc.vector.tensor_tensor(
                out=cand[:],
                in0=eq_hi[:].unsqueeze(2).broadcast_to([P, 8, 8, TOK_PER_PART]),
                in1=eqlo_min[:].unsqueeze(1).broadcast_to([P, 8, 8, TOK_PER_PART]),
                op=mybir.AluOpType.mult)
            red = sb.tile([P, P], F16)
            c3 = cand[:].rearrange("p a b t -> p (a b) t")
            lv1 = big.tile([P, SEG_PER_CORE, 128], F16, tag="lv1")
            nc.vector.tensor_tensor(out=lv1[:], in0=c3[:, :, 0:128],
                                    in1=c3[:, :, 128:256], op=mybir.AluOpType.max)
            lv2 = sb.tile([P, SEG_PER_CORE, 64], F16, tag="lv2")
            nc.vector.tensor_tensor(out=lv2[:], in0=lv1[:, :, 0:64],
                                    in1=lv1[:, :, 64:128], op=mybir.AluOpType.max)
            lv3 = sb.tile([P, SEG_PER_CORE, 32], F16, tag="lv3")
            nc.vector.tensor_tensor(out=lv3[:], in0=lv2[:, :, 0:32],
                                    in1=lv2[:, :, 32:64], op=mybir.AluOpType.max)
            nc.vector.tensor_reduce(red[:, 0:SEG_PER_CORE], lv3[:],
                                    axis=mybir.AxisListType.X,
                                    op=mybir.AluOpType.max)
            cand2 = big.tile([P, 8, 8, TOK_PER_PART], F16)
            nc.vector.tensor_tensor(
                out=cand2[:],
                in0=eq_hi[:].unsqueeze(2).broadcast_to([P, 8, 8, TOK_PER_PART]),
                in1=eqlo_max[:].unsqueeze(1).broadcast_to([P, 8, 8, TOK_PER_PART]),
                op=mybir.AluOpType.mult)
            c3b = cand2[:].rearrange("p a b t -> p (a b) t")
            lv1b = big.tile([P, SEG_PER_CORE, 128], F16, tag="lv1")
            nc.vector.tensor_tensor(out=lv1b[:], in0=c3b[:, :, 0:128],
                                    in1=c3b[:, :, 128:256], op=mybir.AluOpType.max)
            lv2b = sb.tile([P, SEG_PER_CORE, 64], F16, tag="lv2")
            nc.vector.tensor_tensor(out=lv2b[:], in0=lv1b[:, :, 0:64],
                                    in1=lv1b[:, :, 64:128], op=mybir.AluOpType.max)
            lv3b = sb.tile([P, SEG_PER_CORE, 32], F16, tag="lv3")
            nc.vector.tensor_tensor(out=lv3b[:], in0=lv2b[:, :, 0:32],
                                    in1=lv2b[:, :, 32:64], op=mybir.AluOpType.max)
            nc.vector.tensor_reduce(red[:, SEG_PER_CORE:P], lv3b[:],
                                    axis=mybir.AxisListType.X,
                                    op=mybir.AluOpType.max)

            ident = sb.tile([P, P], F16)
            make_identity(nc, ident[:])
            red_t = ps.tile([P, P], F16)
            nc.tensor.transpose(out=red_t[:], in_=red[:], identity=ident[:])
            mask = sb.tile([P, P], F32)
            nc.vector.tensor_scalar(mask[:], red_t[:], 0.0, None,
                                    op0=mybir.AluOpType.is_gt)
            glob = sb.tile([P, P], F32)
            nc.vector.tensor_tensor(out=glob[:], in0=red_t[:], in1=base_t[:],
                                    op=mybir.AluOpType.add)
            nc.vector.tensor_tensor(out=glob[:], in0=glob[:], in1=mask[:],
                                    op=mybir.AluOpType.mult)
            enc = sb.tile([P, 1], F32)
            nc.vector.tensor_reduce(enc[:], glob[:],
                                    axis=mybir.AxisListType.X,
                                    op=mybir.AluOpType.max)
            idx_f = sb.tile([P, 1], F32)
            nc.vector.tensor_scalar(idx_f[0:SEG_PER_CORE, :], enc[0:SEG_PER_CORE, :],
                                    -1.0, float(L),
                                    op0=mybir.AluOpType.mult,
                                    op1=mybir.AluOpType.add)
            nc.vector.tensor_scalar_add(idx_f[SEG_PER_CORE:P, :],
                                        enc[SEG_PER_CORE:P, :], -1.0)
            idx_i = sb.tile([P, 1], I32)
            nc.vector.tensor_copy(idx_i[:], idx_f[:])
            rows = big.tile([P, H], F32)
            nc.gpsimd.indirect_dma_start(
                out=rows[:], out_offset=None, in_=x.ap(),
                in_offset=bass.IndirectOffsetOnAxis(ap=idx_i[:, 0:1], axis=0))
            nc.gpsimd.dma_start(out.ap()[:, 0:H], rows[0:SEG_PER_CORE, :])
            nc.sync.dma_start(out.ap()[:, H:2 * H], rows[SEG_PER_CORE:P, :])

    nc.compile()
    return nc


def make_in_maps_general(input, number_mask):
    x = np.ascontiguousarray(np.asarray(input), dtype=np.float32).reshape(L, H)
    nm = np.ascontiguousarray(np.asarray(number_mask))
    if nm.dtype != np.int64:
        nm = nm.astype(np.int64)
    idpairs = nm.reshape(L).view(np.int32).reshape(P, TOK_PER_PART, 2)
    c8lo = np.repeat(np.arange(8, dtype=np.float16), TOK_PER_PART)
    f = np.arange(TOK_PER_PART, dtype=np.float16)
    pcol = np.arange(P, dtype=np.float32)
    base = np.empty((2, SEG_PER_CORE, P), dtype=np.float32)
    base[0] = (P - 1 - pcol) * TOK_PER_PART
    base[1] = pcol * TOK_PER_PART
    in_maps = []
    for c in range(NCORES):
        c8hi = np.repeat(np.arange(8, dtype=np.float16) + c * 8, TOK_PER_PART)
        cpack = np.tile(np.concatenate([c8hi, c8lo, TOK_PER_PART - f, f + 1]),
                        (P, 1))
        in_maps.append({"x": x, "idpairs": idpairs, "cpack": cpack,
                        "base": base})
    return in_maps


_NC = None
_NC_GENERAL = None


def _get_nc():
    global _NC
    if _NC is None:
        _NC = build_nc()
    return _NC


def _get_nc_general():
    global _NC_GENERAL
    if _NC_GENERAL is None:
        _NC_GENERAL = build_nc_general()
    return _NC_GENERAL


def _rows_distinct(number_mask):
    ids = np.asarray(number_mask).reshape(P, TOK_PER_PART)
    s = np.sort(ids, axis=1)
    return not np.any(s[:, 1:] == s[:, :-1])


def kernel(input, number_mask, n, concat, **_):
    assert int(n) == NSEG and int(concat) == 1
    if _rows_distinct(number_mask):
        nc = _get_nc()
        in_maps = make_in_maps(input, number_mask)
    else:
        nc = _get_nc_general()
        in_maps = make_in_maps_general(input, number_mask)
    res = bass_utils.run_bass_kernel_spmd(nc, in_maps, core_ids=list(range(NCORES)))
    return np.concatenate([res.results[c]["out"] for c in range(NCORES)], axis=0)


# revision 12
# speedup vs baseline: 2.5375x; 1.0166x over previous
"""Trainium2 Bass kernel for nn_AwareDecoder segment first/last gather.

Problem: input [16, 2048, 1024] f32, number_mask [16, 2048] int64 with ids in
[0, 512]. For each segment id i in [0, 512): find first/last row-major token
position with that id, gather those rows of the flattened input, concat ->
out [512, 2048] f32.

Fast path (8 NeuronCores, segment-sharded - no collectives):
  core c owns segments [64c, 64c+64). Token t = (p, f) with partition
  p = t >> 8 and in-row offset f = t & 255. Within one 256-token row every
  occurring id appears at most once (host-verified; true for the reference's
  arange % 513 mask since 256 < 513), so a single GPSIMD local_scatter builds
  the whole per-row segment table in one shot:

    idx16[p, f] = id[p, f] - 64c   (negative / out-of-range ids are ignored
                                    or land in unused table slots)
    tab[p, v]   = f + 1 where id[p, f] == 64c + v, else 0

  The cross-row combine reuses the encode/transpose/decode trick: cols 0:64
  of a [128, 128] tile hold -tab (first side), cols 64:128 hold +tab (last
  side); PE-transpose, add per-side bases ((127-p)*256 + 257 resp. 256p - 1),
  mask zeros, free-axis max-reduce, then one hardware indirect DMA gathers
  the 64+64 rows (512KB of the 128MB input) and two DMAs write the
  [64, 2048] output slice. Host concatenates the 8 slices.

Fallback (any per-row duplicate id): the original eq/select/reduce sweep
kernel, compiled lazily.
"""
import numpy as np

import concourse.bass as bass
import concourse.tile as tile
from concourse import bacc, library_config, mybir
from concourse import bass_utils

P = 128            # partitions
L = 32768          # B*S tokens
H = 1024           # hidden
NSEG = 512         # segments
NCORES = 8
SEG_PER_CORE = NSEG // NCORES            # 64
TOK_PER_PART = L // P                    # 256 tokens per partition
NELEM = 514        # local_scatter table width (>= 513, even)
F32 = mybir.dt.float32
F16 = mybir.dt.float16
I32 = mybir.dt.int32
I16 = mybir.dt.int16


def build_nc():
    nc = bacc.Bacc("TRN2", target_bir_lowering=False, debug=False)

    x = nc.dram_tensor("x", [L, H], F32, kind="ExternalInput")
    # number_mask int64 raw bytes as int32 (lo, hi) pairs; partition p covers
    # tokens [p*256, (p+1)*256).
    idpairs = nc.dram_tensor("idpairs", [P, TOK_PER_PART, 2], I32, kind="ExternalInput")
    # per-core segment base (64*c), replicated over partitions
    segb_in = nc.dram_tensor("segb", [P, 1], F32, kind="ExternalInput")
    # fp16 consts: [identity (128) | enc = f+1 (256)]
    cpk_in = nc.dram_tensor("cpk", [P, P + TOK_PER_PART], F16, kind="ExternalInput")
    # decode bases: rows 0..63  (min side): (127 - p) * 256 + 257
    #               rows 64..127 (max side): 256 * p - 1
    base_in = nc.dram_tensor("base", [P, P], F32, kind="ExternalInput")
    out = nc.dram_tensor("out", [SEG_PER_CORE, 2 * H], F32, kind="ExternalOutput")

    with tile.TileContext(nc) as tc:
        with tc.tile_pool(name="sb", bufs=1) as sb, \
             tc.tile_pool(name="ps", bufs=1, space="PSUM") as ps:

            # hoist the GPSIMD library swap off the critical path: its ucode
            # DMA (~2.4us) then overlaps the input DMAs
            nc.gpsimd.load_library(library_config.local_scatter)

            segb_t = sb.tile([P, 1], F32)
            nc.scalar.dma_start(segb_t[:], segb_in.ap())
            idp_t = sb.tile([P, TOK_PER_PART, 2], I32)
            nc.sync.dma_start(idp_t[0:P // 2], idpairs.ap()[0:P // 2])
            nc.scalar.dma_start(idp_t[P // 2:P], idpairs.ap()[P // 2:P])
            cpk_t = sb.tile([P, P + TOK_PER_PART], F16)
            nc.scalar.dma_start(cpk_t[:], cpk_in.ap())
            base_t = sb.tile([P, P], F32)
            nc.sync.dma_start(base_t[:], base_in.ap())
            ident = cpk_t[:, 0:P]
            enc = cpk_t[:, P:P + TOK_PER_PART]

            # rebase ids to the core's segment range and narrow to int16
            idx16 = sb.tile([P, TOK_PER_PART], I16)
            nc.vector.tensor_scalar(idx16[:], idp_t[:, :, 0], segb_t[:, 0:1], None,
                                    op0=mybir.AluOpType.subtract)

            # one scatter builds the whole per-row segment table
            tab = sb.tile([P, NELEM], F16)
            nc.gpsimd.local_scatter(tab[:], enc, idx16[:],
                                    channels=P, num_elems=NELEM,
                                    num_idxs=TOK_PER_PART)

            # [:, 0:64] = -tab (first side), [:, 64:128] = +tab (last side)
            red = sb.tile([P, P], F16)
            nc.vector.tensor_scalar(red[:, 0:SEG_PER_CORE], tab[:, 0:SEG_PER_CORE],
                                    -1.0, None, op0=mybir.AluOpType.mult)
            nc.vector.tensor_copy(red[:, SEG_PER_CORE:P], tab[:, 0:SEG_PER_CORE])

            # cross-partition combine: transpose, add bases, mask, max-reduce
            red_t = ps.tile([P, P], F16)
            nc.tensor.transpose(out=red_t[:], in_=red[:], identity=ident)
            summ = sb.tile([P, P], F32)
            nc.vector.tensor_tensor(out=summ[:], in0=red_t[:], in1=base_t[:],
                                    op=mybir.AluOpType.add)
            glob = sb.tile([P, P], F32)
            nc.vector.scalar_tensor_tensor(out=glob[:], in0=red_t[:], scalar=0.0,
                                           in1=summ[:],
                                           op0=mybir.AluOpType.not_equal,
                                           op1=mybir.AluOpType.mult)
            enc1 = sb.tile([P, 1], F32)
            nc.vector.tensor_reduce(enc1[:], glob[:],
                                    axis=mybir.AxisListType.X,
                                    op=mybir.AluOpType.max)
            # decode: first = L - enc, last = enc
            idx_f = sb.tile([P, 1], F32)
            nc.vector.tensor_scalar(idx_f[0:SEG_PER_CORE, :], enc1[0:SEG_PER_CORE, :],
                                    -1.0, float(L),
                                    op0=mybir.AluOpType.mult,
                                    op1=mybir.AluOpType.add)
            nc.vector.tensor_copy(idx_f[SEG_PER_CORE:P, :], enc1[SEG_PER_CORE:P, :])
            idx_i = sb.tile([P, 1], I32)
            nc.vector.tensor_copy(idx_i[:], idx_f[:])

            rows = sb.tile([P, H], F32)
            nc.gpsimd.indirect_dma_start(
                out=rows[:], out_offset=None, in_=x.ap(),
                in_offset=bass.IndirectOffsetOnAxis(ap=idx_i[:, 0:1], axis=0))
            nc.sync.dma_start(out.ap()[:, 0:H], rows[0:SEG_PER_CORE, :])
            nc.scalar.dma_start(out.ap()[:, H:2 * H], rows[SEG_PER_CORE:P, :])

    nc.compile()
    return nc


def make_in_maps(input, number_mask):
    x = np.ascontiguousarray(np.asarray(input), dtype=np.float32).reshape(L, H)
    nm = np.ascontiguousarray(np.asarray(number_mask))
    if nm.dtype != np.int64:
        nm = nm.astype(np.int64)
    idpairs = nm.reshape(L).view(np.int32).reshape(P, TOK_PER_PART, 2)
    ident = np.eye(P, dtype=np.float16)
    enc = np.tile(np.arange(1, TOK_PER_PART + 1, dtype=np.float16), (P, 1))
    cpk = np.concatenate([ident, enc], axis=1)
    pcol = np.arange(P, dtype=np.float32)
    base = np.empty((P, P), dtype=np.float32)
    base[0:SEG_PER_CORE] = (P - 1 - pcol) * TOK_PER_PART + TOK_PER_PART + 1
    base[SEG_PER_CORE:P] = pcol * TOK_PER_PART - 1
    in_maps = []
    for c in range(NCORES):
        segb = np.full((P, 1), c * SEG_PER_CORE, dtype=np.float32)
        in_maps.append({"x": x, "idpairs": idpairs, "segb": segb,
                        "cpk": cpk, "base": base})
    return in_maps


# ---------------------------------------------------------------------------
# Fallback: original eq/select/reduce sweep (handles per-row duplicate ids).
# ---------------------------------------------------------------------------

def build_nc_general():
    from concourse.masks import make_identity

    nc = bacc.Bacc("TRN2", target_bir_lowering=False, debug=False)

    x = nc.dram_tensor("x", [L, H], F32, kind="ExternalInput")
    idpairs = nc.dram_tensor("idpairs", [P, TOK_PER_PART, 2], I32, kind="ExternalInput")
    cpack_in = nc.dram_tensor("cpack", [P, 18 * TOK_PER_PART], F16,
                              kind="ExternalInput")
    base_in = nc.dram_tensor("base", [2, SEG_PER_CORE, P], F32, kind="ExternalInput")
    out = nc.dram_tensor("out", [SEG_PER_CORE, 2 * H], F32, kind="ExternalOutput")

    with tile.TileContext(nc) as tc:
        with tc.tile_pool(name="sb", bufs=1) as sb, \
             tc.tile_pool(name="big", bufs=1) as big, \
             tc.tile_pool(name="ps", bufs=1, space="PSUM") as ps:

            idp_t = sb.tile([P, TOK_PER_PART, 2], I32)
            nc.sync.dma_start(idp_t[:], idpairs.ap())
            cpack = sb.tile([P, 18 * TOK_PER_PART], F16)
            nc.scalar.dma_start(cpack[:], cpack_in.ap())
            c8hi_t = cpack[:, 0:8 * TOK_PER_PART].rearrange(
                "p (a t) -> p a t", a=8)
            c8lo_t = cpack[:, 8 * TOK_PER_PART:16 * TOK_PER_PART].rearrange(
                "p (a t) -> p a t", a=8)
            posmin = cpack[:, 16 * TOK_PER_PART:17 * TOK_PER_PART]
            posmax = cpack[:, 17 * TOK_PER_PART:18 * TOK_PER_PART]
            base_t = sb.tile([P, P], F32)
            nc.gpsimd.dma_start(base_t[:], base_in.ap().rearrange("a s p -> (a s) p"))

            hi_i = sb.tile([P, TOK_PER_PART], I32)
            nc.vector.tensor_scalar(hi_i[:], idp_t[:, :, 0], 3, None,
                                    op0=mybir.AluOpType.arith_shift_right)
            lo_i = sb.tile([P, TOK_PER_PART], I32)
            nc.vector.tensor_scalar(lo_i[:], idp_t[:, :, 0], 7, None,
                                    op0=mybir.AluOpType.bitwise_and)
            hi_f = sb.tile([P, TOK_PER_PART], F16)
            nc.vector.tensor_copy(hi_f[:], hi_i[:])
            lo_f = sb.tile([P, TOK_PER_PART], F16)
            nc.vector.tensor_copy(lo_f[:], lo_i[:])

            eq_hi = sb.tile([P, 8, TOK_PER_PART], F16)
            nc.vector.tensor_tensor(
                out=eq_hi[:],
                in0=hi_f[:].unsqueeze(1).broadcast_to([P, 8, TOK_PER_PART]),
                in1=c8hi_t, op=mybir.AluOpType.is_equal)
            eq_lo = sb.tile([P, 8, TOK_PER_PART], F16)
            nc.vector.tensor_tensor(
                out=eq_lo[:],
                in0=lo_f[:].unsqueeze(1).broadcast_to([P, 8, TOK_PER_PART]),
                in1=c8lo_t, op=mybir.AluOpType.is_equal)
            eqlo_min = sb.tile([P, 8, TOK_PER_PART], F16)
            nc.vector.tensor_tensor(
                out=eqlo_min[:], in0=eq_lo[:],
                in1=posmin.unsqueeze(1).broadcast_to([P, 8, TOK_PER_PART]),
                op=mybir.AluOpType.mult)
            eqlo_max = sb.tile([P, 8, TOK_PER_PART], F16)
            nc.vector.tensor_tensor(
                out=eqlo_max[:], in0=eq_lo[:],
                in1=posmax.unsqueeze(1).broadcast_to([P, 8, TOK_PER_PART]),
                op=mybir.AluOpType.mult)

            cand = big.tile([P, 8, 8, TOK_PER_PART], F16)
            nc.vector.tensor_tensor(
                out=cand[:],
                in0=eq_hi[:].unsqueeze(2).broadcast_to([P, 8, 8, TOK_PER_PART]),
                in1=eqlo_min[:].unsqueeze(1).broadcast_to([P, 8, 8, TOK_PER_PART]),
                op=mybir.AluOpType.mult)
            red = sb.tile([P, P], F16)
            c3 = cand[:].rearrange("p a b t -> p (a b) t")
            lv1 = big.tile([P, SEG_PER_CORE, 128], F16, tag="lv1")
            nc.vector.tensor_tensor(out=lv1[:], in0=c3[:, :, 0:128],
                                    in1=c3[:, :, 128:256], op=mybir.AluOpType.max)
            lv2 = sb.tile([P, SEG_PER_CORE, 64], F16, tag="lv2")
            nc.vector.tensor_tensor(out=lv2[:], in0=lv1[:, :, 0:64],
                                    in1=lv1[:, :, 64:128], op=mybir.AluOpType.max)
            lv3 = sb.tile([P, SEG_PER_CORE, 32], F16, tag="lv3")
            nc.vector.tensor_tensor(out=lv3[:], in0=lv2[:, :, 0:32],
                                    in1=lv2[:, :, 32:64], op=mybir.AluOpType.max)
            nc.vector.tensor_reduce(red[:, 0:SEG_PER_CORE], lv3[:],
                                    axis=mybir.AxisListType.X,
                                    op=mybir.AluOpType.max)
            cand2 = big.tile([P, 8, 8, TOK_PER_PART], F16)
            nc.vector.tensor_tensor(
                out=cand2[:],
                in0=eq_hi[:].unsqueeze(2).broadcast_to([P, 8, 8, TOK_PER_PART]),
                in1=eqlo_max[:].unsqueeze(1).broadcast_to([P, 8, 8, TOK_PER_PART]),
                op=mybir.AluOpType.mult)
            c3b = cand2[:].rearrange("p a b t -> p (a b) t")
            lv1b = big.tile([P, SEG_PER_CORE, 128], F16, tag="lv1")
            nc.vector.tensor_tensor(out=lv1b[:], in0=c3b[:, :, 0:128],
                                    in1=c3b[:, :, 128:256], op=mybir.AluOpType.max)
            lv2b = sb.tile([P, SEG_PER_CORE, 64], F16, tag="lv2")
            nc.vector.tensor_tensor(out=lv2b[:], in0=lv1b[:, :, 0:64],
                                    in1=lv1b[:, :, 64:128], op=mybir.AluOpType.max)
            lv3b = sb.tile([P, SEG_PER_CORE, 32], F16, tag="lv3")
            nc.vector.tensor_tensor(out=lv3b[:], in0=lv2b[:, :, 0:32],
                                    in1=lv2b[:, :, 32:64], op=mybir.AluOpType.max)
            nc.vector.tensor_reduce(red[:, SEG_PER_CORE:P], lv3b[:],
                                    axis=mybir.AxisListType.X,
                                    op=mybir.AluOpType.max)

            ident = sb.tile([P, P], F16)
            make_identity(nc, ident[:])
            red_t = ps.tile([P, P], F16)
            nc.tensor.transpose(out=red_t[:], in_=red[:], identity=ident[:])
            mask = sb.tile([P, P], F32)
            nc.vector.tensor_scalar(mask[:], red_t[:], 0.0, None,
                                    op0=mybir.AluOpType.is_gt)
            glob = sb.tile([P, P], F32)
            nc.vector.tensor_tensor(out=glob[:], in0=red_t[:], in1=base_t[:],
                                    op=mybir.AluOpType.add)
            nc.vector.tensor_tensor(out=glob[:], in0=glob[:], in1=mask[:],
                                    op=mybir.AluOpType.mult)
            enc = sb.tile([P, 1], F32)
            nc.vector.tensor_reduce(enc[:], glob[:],
                                    axis=mybir.AxisListType.X,
                                    op=mybir.AluOpType.max)
            idx_f = sb.tile([P, 1], F32)
            nc.vector.tensor_scalar(idx_f[0:SEG_PER_CORE, :], enc[0:SEG_PER_CORE, :],
                                    -1.0, float(L),
                                    op0=mybir.AluOpType.mult,
                                    op1=mybir.AluOpType.add)
            nc.vector.tensor_scalar_add(idx_f[SEG_PER_CORE:P, :],
                                        enc[SEG_PER_CORE:P, :], -1.0)
            idx_i = sb.tile([P, 1], I32)
            nc.vector.tensor_copy(idx_i[:], idx_f[:])
            rows = big.tile([P, H], F32)
            nc.gpsimd.indirect_dma_start(
                out=rows[:], out_offset=None, in_=x.ap(),
                in_offset=bass.IndirectOffsetOnAxis(ap=idx_i[:, 0:1], axis=0))
            nc.gpsimd.dma_start(out.ap()[:, 0:H], rows[0:SEG_PER_CORE, :])
            nc.sync.dma_start(out.ap()[:, H:2 * H], rows[SEG_PER_CORE:P, :])

    nc.compile()
    return nc


def make_in_maps_general(input, number_mask):
    x = np.ascontiguousarray(np.asarray(input), dtype=np.float32).reshape(L, H)
    nm = np.ascontiguousarray(np.asarray(number_mask))
    if nm.dtype != np.int64:
        nm = nm.astype(np.int64)
    idpairs = nm.reshape(L).view(np.int32).reshape(P, TOK_PER_PART, 2)
    c8lo = np.repeat(np.arange(8, dtype=np.float16), TOK_PER_PART)
    f = np.arange(TOK_PER_PART, dtype=np.float16)
    pcol = np.arange(P, dtype=np.float32)
    base = np.empty((2, SEG_PER_CORE, P), dtype=np.float32)
    base[0] = (P - 1 - pcol) * TOK_PER_PART
    base[1] = pcol * TOK_PER_PART
    in_maps = []
    for c in range(NCORES):
        c8hi = np.repeat(np.arange(8, dtype=np.float16) + c * 8, TOK_PER_PART)
        cpack = np.tile(np.concatenate([c8hi, c8lo, TOK_PER_PART - f, f + 1]),
                        (P, 1))
        in_maps.append({"x": x, "idpairs": idpairs, "cpack": cpack,
                        "base": base})
    return in_maps


_NC = None
_NC_GENERAL = None


def _get_nc():
    global _NC
    if _NC is None:
        _NC = build_nc()
    return _NC


def _get_nc_general():
    global _NC_GENERAL
    if _NC_GENERAL is None:
        _NC_GENERAL = build_nc_general()
    return _NC_GENERAL


def _rows_distinct(number_mask):
    ids = np.asarray(number_mask).reshape(P, TOK_PER_PART)
    s = np.sort(ids, axis=1)
    return not np.any(s[:, 1:] == s[:, :-1])


def kernel(input, number_mask, n, concat, **_):
    assert int(n) == NSEG and int(concat) == 1
    if _rows_distinct(number_mask):
        nc = _get_nc()
        in_maps = make_in_maps(input, number_mask)
    else:
        nc = _get_nc_general()
        in_maps = make_in_maps_general(input, number_mask)
    res = bass_utils.run_bass_kernel_spmd(nc, in_maps, core_ids=list(range(NCORES)))
    return np.concatenate([res.results[c]["out"] for c in range(NCORES)], axis=0)


# revision 17
# speedup vs baseline: 2.5422x; 1.0019x over previous
"""Trainium2 Bass kernel for nn_AwareDecoder segment first/last gather.

Problem: input [16, 2048, 1024] f32, number_mask [16, 2048] int64 with ids in
[0, 512]. For each segment id i in [0, 512): find first/last row-major token
position with that id, gather those rows of the flattened input, concat ->
out [512, 2048] f32.

Fast path (8 NeuronCores, segment-sharded - no collectives):
  core c owns segments [64c, 64c+64). Token t = (p, f) with partition
  p = t >> 8 and in-row offset f = t & 255. Within one 256-token row every
  occurring id appears at most once (host-verified; true for the reference's
  arange % 513 mask since 256 < 513), so a single GPSIMD local_scatter builds
  the whole per-row segment table in one shot:

    idx16[p, f] = id[p, f] - 64c   (negative / out-of-range ids are ignored
                                    or land in unused table slots)
    tab[p, v]   = f + 1 where id[p, f] == 64c + v, else 0

  The cross-row combine reuses the encode/transpose/decode trick: cols 0:64
  of a [128, 128] tile hold -tab (first side), cols 64:128 hold +tab (last
  side); PE-transpose, add per-side bases ((127-p)*256 + 257 resp. 256p - 1),
  mask zeros, free-axis max-reduce, then one hardware indirect DMA gathers
  the 64+64 rows (512KB of the 128MB input) and two DMAs write the
  [64, 2048] output slice. Host concatenates the 8 slices.

Fallback (any per-row duplicate id): the original eq/select/reduce sweep
kernel, compiled lazily.
"""
import numpy as np

import concourse.bass as bass
import concourse.tile as tile
from concourse import bacc, library_config, mybir
from concourse import bass_utils

P = 128            # partitions
L = 32768          # B*S tokens
H = 1024           # hidden
NSEG = 512         # segments
NCORES = 8
SEG_PER_CORE = NSEG // NCORES            # 64
TOK_PER_PART = L // P                    # 256 tokens per partition
NELEM = 514        # local_scatter table width (>= 513, even)
F32 = mybir.dt.float32
F16 = mybir.dt.float16
I32 = mybir.dt.int32
I16 = mybir.dt.int16


def build_nc():
    nc = bacc.Bacc("TRN2", target_bir_lowering=False, debug=False)

    x = nc.dram_tensor("x", [L, H], F32, kind="ExternalInput")
    # number_mask int64 raw bytes as int32 (lo, hi) pairs; partition p covers
    # tokens [p*256, (p+1)*256).
    idpairs = nc.dram_tensor("idpairs", [P, TOK_PER_PART, 2], I32, kind="ExternalInput")
    # per-core segment base (64*c), replicated over partitions
    segb_in = nc.dram_tensor("segb", [P, 1], F32, kind="ExternalInput")
    # fp16 consts, split so enc can land early on its own queue
    enc_in = nc.dram_tensor("encc", [P, TOK_PER_PART], F16, kind="ExternalInput")
    ident_in = nc.dram_tensor("ident", [P, P], F16, kind="ExternalInput")
    # decode bases: rows 0..63  (min side): (127 - p) * 256 + 257
    #               rows 64..127 (max side): 256 * p - 1
    base_in = nc.dram_tensor("base", [P, P], F32, kind="ExternalInput")
    out = nc.dram_tensor("out", [SEG_PER_CORE, 2 * H], F32, kind="ExternalOutput")

    with tile.TileContext(nc) as tc:
        with tc.tile_pool(name="sb", bufs=1) as sb, \
             tc.tile_pool(name="ps", bufs=1, space="PSUM") as ps:

            # hoist the GPSIMD library swap off the critical path: its ucode
            # DMA (~2.4us) then overlaps the input DMAs
            nc.gpsimd.load_library(library_config.local_scatter)

            segb_t = sb.tile([P, 1], F32)
            nc.scalar.dma_start(segb_t[:], segb_in.ap())
            idp_t = sb.tile([P, TOK_PER_PART, 2], I32)
            nc.sync.dma_start(idp_t[0:P // 2], idpairs.ap()[0:P // 2])
            nc.scalar.dma_start(idp_t[P // 2:P], idpairs.ap()[P // 2:P])
            enc = sb.tile([P, TOK_PER_PART], F16)
            nc.sync.dma_start(enc[:], enc_in.ap())
            ident = sb.tile([P, P], F16)
            nc.scalar.dma_start(ident[:], ident_in.ap())
            base_t = sb.tile([P, P], F32)
            nc.scalar.dma_start(base_t[:], base_in.ap())

            # rebase ids to the core's segment range and narrow to int16
            idx16 = sb.tile([P, TOK_PER_PART], I16)
            nc.vector.tensor_scalar(idx16[:], idp_t[:, :, 0], segb_t[:, 0:1], None,
                                    op0=mybir.AluOpType.subtract)

            # one scatter builds the whole per-row segment table
            tab = sb.tile([P, NELEM], F16)
            nc.gpsimd.local_scatter(tab[:], enc[:], idx16[:],
                                    channels=P, num_elems=NELEM,
                                    num_idxs=TOK_PER_PART)

            # [:, 0:64] = -tab (first side), [:, 64:128] = +tab (last side)
            red = sb.tile([P, P], F16)
            nc.vector.tensor_scalar(red[:, 0:SEG_PER_CORE], tab[:, 0:SEG_PER_CORE],
                                    -1.0, None, op0=mybir.AluOpType.mult)
            nc.vector.tensor_copy(red[:, SEG_PER_CORE:P], tab[:, 0:SEG_PER_CORE])

            # cross-partition combine: transpose, add bases, mask, max-reduce
            red_t = ps.tile([P, P], F16)
            nc.tensor.transpose(out=red_t[:], in_=red[:], identity=ident[:])
            summ = sb.tile([P, P], F32)
            nc.vector.tensor_tensor(out=summ[:], in0=red_t[:], in1=base_t[:],
                                    op=mybir.AluOpType.add)
            glob = sb.tile([P, P], F32)
            nc.vector.scalar_tensor_tensor(out=glob[:], in0=red_t[:], scalar=0.0,
                                           in1=summ[:],
                                           op0=mybir.AluOpType.not_equal,
                                           op1=mybir.AluOpType.mult)
            enc1 = sb.tile([P, 1], F32)
            nc.vector.tensor_reduce(enc1[:], glob[:],
                                    axis=mybir.AxisListType.X,
                                    op=mybir.AluOpType.max)
            # decode: first = L - enc, last = enc
            idx_f = sb.tile([P, 1], F32)
            nc.vector.tensor_scalar(idx_f[0:SEG_PER_CORE, :], enc1[0:SEG_PER_CORE, :],
                                    -1.0, float(L),
                                    op0=mybir.AluOpType.mult,
                                    op1=mybir.AluOpType.add)
            nc.vector.tensor_copy(idx_f[SEG_PER_CORE:P, :], enc1[SEG_PER_CORE:P, :])
            idx_i = sb.tile([P, 1], I32)
            nc.vector.tensor_copy(idx_i[:], idx_f[:])

            rows = sb.tile([P, H], F32)
            nc.gpsimd.indirect_dma_start(
                out=rows[:], out_offset=None, in_=x.ap(),
                in_offset=bass.IndirectOffsetOnAxis(ap=idx_i[:, 0:1], axis=0))
            nc.sync.dma_start(out.ap()[:, 0:H], rows[0:SEG_PER_CORE, :])
            nc.scalar.dma_start(out.ap()[:, H:2 * H], rows[SEG_PER_CORE:P, :])

    nc.compile()
    return nc


def make_in_maps(input, number_mask):
    x = np.ascontiguousarray(np.asarray(input), dtype=np.float32).reshape(L, H)
    nm = np.ascontiguousarray(np.asarray(number_mask))
    if nm.dtype != np.int64:
        nm = nm.astype(np.int64)
    idpairs = nm.reshape(L).view(np.int32).reshape(P, TOK_PER_PART, 2)
    ident = np.eye(P, dtype=np.float16)
    enc = np.tile(np.arange(1, TOK_PER_PART + 1, dtype=np.float16), (P, 1))
    pcol = np.arange(P, dtype=np.float32)
    base = np.empty((P, P), dtype=np.float32)
    base[0:SEG_PER_CORE] = (P - 1 - pcol) * TOK_PER_PART + TOK_PER_PART + 1
    base[SEG_PER_CORE:P] = pcol * TOK_PER_PART - 1
    in_maps = []
    for c in range(NCORES):
        segb = np.full((P, 1), c * SEG_PER_CORE, dtype=np.float32)
        in_maps.append({"x": x, "idpairs": idpairs, "segb": segb,
                        "encc": enc, "ident": ident, "base": base})
    return in_maps


# ---------------------------------------------------------------------------
# Fallback: original eq/select/reduce sweep (handles per-row duplicate ids).
# ---------------------------------------------------------------------------

def build_nc_general():
    from concourse.masks import make_identity

    nc = bacc.Bacc("TRN2", target_bir_lowering=False, debug=False)

    x = nc.dram_tensor("x", [L, H], F32, kind="ExternalInput")
    idpairs = nc.dram_tensor("idpairs", [P, TOK_PER_PART, 2], I32, kind="ExternalInput")
    cpack_in = nc.dram_tensor("cpack", [P, 18 * TOK_PER_PART], F16,
                              kind="ExternalInput")
    base_in = nc.dram_tensor("base", [2, SEG_PER_CORE, P], F32, kind="ExternalInput")
    out = nc.dram_tensor("out", [SEG_PER_CORE, 2 * H], F32, kind="ExternalOutput")

    with tile.TileContext(nc) as tc:
        with tc.tile_pool(name="sb", bufs=1) as sb, \
             tc.tile_pool(name="big", bufs=1) as big, \
             tc.tile_pool(name="ps", bufs=1, space="PSUM") as ps:

            idp_t = sb.tile([P, TOK_PER_PART, 2], I32)
            nc.sync.dma_start(idp_t[:], idpairs.ap())
            cpack = sb.tile([P, 18 * TOK_PER_PART], F16)
            nc.scalar.dma_start(cpack[:], cpack_in.ap())
            c8hi_t = cpack[:, 0:8 * TOK_PER_PART].rearrange(
                "p (a t) -> p a t", a=8)
            c8lo_t = cpack[:, 8 * TOK_PER_PART:16 * TOK_PER_PART].rearrange(
                "p (a t) -> p a t", a=8)
            posmin = cpack[:, 16 * TOK_PER_PART:17 * TOK_PER_PART]
            posmax = cpack[:, 17 * TOK_PER_PART:18 * TOK_PER_PART]
            base_t = sb.tile([P, P], F32)
            nc.gpsimd.dma_start(base_t[:], base_in.ap().rearrange("a s p -> (a s) p"))

            hi_i = sb.tile([P, TOK_PER_PART], I32)
            nc.vector.tensor_scalar(hi_i[:], idp_t[:, :, 0], 3, None,
                                    op0=mybir.AluOpType.arith_shift_right)
            lo_i = sb.tile([P, TOK_PER_PART], I32)
            nc.vector.tensor_scalar(lo_i[:], idp_t[:, :, 0], 7, None,
                                    op0=mybir.AluOpType.bitwise_and)
            hi_f = sb.tile([P, TOK_PER_PART], F16)
            nc.vector.tensor_copy(hi_f[:], hi_i[:])
            lo_f = sb.tile([P, TOK_PER_PART], F16)
            nc.vector.tensor_copy(lo_f[:], lo_i[:])

            eq_hi = sb.tile([P, 8, TOK_PER_PART], F16)
            nc.vector.tensor_tensor(
                out=eq_hi[:],
                in0=hi_f[:].unsqueeze(1).broadcast_to([P, 8, TOK_PER_PART]),
                in1=c8hi_t, op=mybir.AluOpType.is_equal)
            eq_lo = sb.tile([P, 8, TOK_PER_PART], F16)
            nc.vector.tensor_tensor(
                out=eq_lo[:],
                in0=lo_f[:].unsqueeze(1).broadcast_to([P, 8, TOK_PER_PART]),
                in1=c8lo_t, op=mybir.AluOpType.is_equal)
            eqlo_min = sb.tile([P, 8, TOK_PER_PART], F16)
            nc.vector.tensor_tensor(
                out=eqlo_min[:], in0=eq_lo[:],
                in1=posmin.unsqueeze(1).broadcast_to([P, 8, TOK_PER_PART]),
                op=mybir.AluOpType.mult)
            eqlo_max = sb.tile([P, 8, TOK_PER_PART], F16)
            nc.vector.tensor_tensor(
                out=eqlo_max[:], in0=eq_lo[:],
                in1=posmax.unsqueeze(1).broadcast_to([P, 8, TOK_PER_PART]),
                op=mybir.AluOpType.mult)

            cand = big.tile([P, 8, 8, TOK_PER_PART], F16)
            nc.vector.tensor_tensor(
                out=cand[:],
                in0=eq_hi[:].unsqueeze(2).broadcast_to([P, 8, 8, TOK_PER_PART]),
                in1=eqlo_min[:].unsqueeze(1).broadcast_to([P, 8, 8, TOK_PER_PART]),
                op=mybir.AluOpType.mult)
            red = sb.tile([P, P], F16)
            c3 = cand[:].rearrange("p a b t -> p (a b) t")
            lv1 = big.tile([P, SEG_PER_CORE, 128], F16, tag="lv1")
            nc.vector.tensor_tensor(out=lv1[:], in0=c3[:, :, 0:128],
                                    in1=c3[:, :, 128:256], op=mybir.AluOpType.max)
            lv2 = sb.tile([P, SEG_PER_CORE, 64], F16, tag="lv2")
            nc.vector.tensor_tensor(out=lv2[:], in0=lv1[:, :, 0:64],
                                    in1=lv1[:, :, 64:128], op=mybir.AluOpType.max)
            lv3 = sb.tile([P, SEG_PER_CORE, 32], F16, tag="lv3")
            nc.vector.tensor_tensor(out=lv3[:], in0=lv2[:, :, 0:32],
                                    in1=lv2[:, :, 32:64], op=mybir.AluOpType.max)
            nc.vector.tensor_reduce(red[:, 0:SEG_PER_CORE], lv3[:],
                                    axis=mybir.AxisListType.X,
                                    op=mybir.AluOpType.max)
            cand2 = big.tile([P, 8, 8, TOK_PER_PART], F16)
            nc.vector.tensor_tensor(
                out=cand2[:],
                in0=eq_hi[:].unsqueeze(2).broadcast_to([P, 8, 8, TOK_PER_PART]),
                in1=eqlo_max[:].unsqueeze(1).broadcast_to([P, 8, 8, TOK_PER_PART]),
                op=mybir.AluOpType.mult)
            c3b = cand2[:].rearrange("p a b t -> p (a b) t")
            lv1b = big.tile([P, SEG_PER_CORE, 128], F16, tag="lv1")
            nc.vector.tensor_tensor(out=lv1b[:], in0=c3b[:, :, 0:128],
                                    in1=c3b[:, :, 128:256], op=mybir.AluOpType.max)
            lv2b = sb.tile([P, SEG_PER_CORE, 64], F16, tag="lv2")
            nc.vector.tensor_tensor(out=lv2b[:], in0=lv1b[:, :, 0:64],
                                    in1=lv1b[:, :, 64:128], op=mybir.AluOpType.max)
            lv3b = sb.tile([P, SEG_PER_CORE, 32], F16, tag="lv3")
            nc.vector.tensor_tensor(out=lv3b[:], in0=lv2b[:, :, 0:32],
                                    in1=lv2b[:, :, 32:64], op=mybir.AluOpType.max)
            nc.vector.tensor_reduce(red[:, SEG_PER_CORE:P], lv3b[:],
                                    axis=mybir.AxisListType.X,
                                    op=mybir.AluOpType.max)

            ident = sb.tile([P, P], F16)
            make_identity(nc, ident[:])
            red_t = ps.tile([P, P], F16)
            nc.tensor.transpose(out=red_t[:], in_=red[:], identity=ident[:])
            mask = sb.tile([P, P], F32)
            nc.vector.tensor_scalar(mask[:], red_t[:], 0.0, None,
                                    op0=mybir.AluOpType.is_gt)
            glob = sb.tile([P, P], F32)
            nc.vector.tensor_tensor(out=glob[:], in0=red_t[:], in1=base_t[:],
                                    op=mybir.AluOpType.add)
            nc.vector.tensor_tensor(out=glob[:], in0=glob[:], in1=mask[:],
                                    op=mybir.AluOpType.mult)
            enc = sb.tile([P, 1], F32)
            nc.vector.tensor_reduce(enc[:], glob[:],
                                    axis=mybir.AxisListType.X,
                                    op=mybir.AluOpType.max)
            idx_f = sb.tile([P, 1], F32)
            nc.vector.tensor_scalar(idx_f[0:SEG_PER_CORE, :], enc[0:SEG_PER_CORE, :],
                                    -1.0, float(L),
                                    op0=mybir.AluOpType.mult,
                                    op1=mybir.AluOpType.add)
            nc.vector.tensor_scalar_add(idx_f[SEG_PER_CORE:P, :],
                                        enc[SEG_PER_CORE:P, :], -1.0)
            idx_i = sb.tile([P, 1], I32)
            nc.vector.tensor_copy(idx_i[:], idx_f[:])
            rows = big.tile([P, H], F32)
            nc.gpsimd.indirect_dma_start(
                out=rows[:], out_offset=None, in_=x.ap(),
                in_offset=bass.IndirectOffsetOnAxis(ap=idx_i[:, 0:1], axis=0))
            nc.gpsimd.dma_start(out.ap()[:, 0:H], rows[0:SEG_PER_CORE, :])
            nc.sync.dma_start(out.ap()[:, H:2 * H], rows[SEG_PER_CORE:P, :])

    nc.compile()
    return nc


def make_in_maps_general(input, number_mask):
    x = np.ascontiguousarray(np.asarray(input), dtype=np.float32).reshape(L, H)
    nm = np.ascontiguousarray(np.asarray(number_mask))
    if nm.dtype != np.int64:
        nm = nm.astype(np.int64)
    idpairs = nm.reshape(L).view(np.int32).reshape(P, TOK_PER_PART, 2)
    c8lo = np.repeat(np.arange(8, dtype=np.float16), TOK_PER_PART)
    f = np.arange(TOK_PER_PART, dtype=np.float16)
    pcol = np.arange(P, dtype=np.float32)
    base = np.empty((2, SEG_PER_CORE, P), dtype=np.float32)
    base[0] = (P - 1 - pcol) * TOK_PER_PART
    base[1] = pcol * TOK_PER_PART
    in_maps = []
    for c in range(NCORES):
        c8hi = np.repeat(np.arange(8, dtype=np.float16) + c * 8, TOK_PER_PART)
        cpack = np.tile(np.concatenate([c8hi, c8lo, TOK_PER_PART - f, f + 1]),
                        (P, 1))
        in_maps.append({"x": x, "idpairs": idpairs, "cpack": cpack,
                        "base": base})
    return in_maps


_NC = None
_NC_GENERAL = None


def _get_nc():
    global _NC
    if _NC is None:
        _NC = build_nc()
    return _NC


def _get_nc_general():
    global _NC_GENERAL
    if _NC_GENERAL is None:
        _NC_GENERAL = build_nc_general()
    return _NC_GENERAL


def _rows_distinct(number_mask):
    ids = np.asarray(number_mask).reshape(P, TOK_PER_PART)
    s = np.sort(ids, axis=1)
    return not np.any(s[:, 1:] == s[:, :-1])


def kernel(input, number_mask, n, concat, **_):
    assert int(n) == NSEG and int(concat) == 1
    if _rows_distinct(number_mask):
        nc = _get_nc()
        in_maps = make_in_maps(input, number_mask)
    else:
        nc = _get_nc_general()
        in_maps = make_in_maps_general(input, number_mask)
    res = bass_utils.run_bass_kernel_spmd(nc, in_maps, core_ids=list(range(NCORES)))
    return np.concatenate([res.results[c]["out"] for c in range(NCORES)], axis=0)


# revision 25
# speedup vs baseline: 2.7309x; 1.0742x over previous
"""Trainium2 Bass kernel for nn_AwareDecoder segment first/last gather.

Problem: input [16, 2048, 1024] f32, number_mask [16, 2048] int64 with ids in
[0, 512]. For each segment id i in [0, 512): find first/last row-major token
position with that id, gather those rows of the flattened input, concat ->
out [512, 2048] f32.

Fast path (8 NeuronCores, segment-sharded - no collectives):
  core c owns segments [64c, 64c+64). Token t = (p, f) with partition
  p = t >> 8 and in-row offset f = t & 255. Within one 256-token row every
  occurring id appears at most once (host-verified; true for the reference's
  arange % 513 mask since 256 < 513), so a single GPSIMD local_scatter builds
  the whole per-row segment table in one shot:

    idx16[p, f] = id[p, f] - 64c   (negative / out-of-range ids are ignored
                                    or land in unused table slots)
    tab[p, v]   = f + 1 where id[p, f] == 64c + v, else 0

  The cross-row combine reuses the encode/transpose/decode trick: cols 0:64
  of a [128, 128] tile hold -tab (first side), cols 64:128 hold +tab (last
  side); PE-transpose, add per-side bases ((127-p)*256 + 257 resp. 256p - 1),
  mask zeros, free-axis max-reduce, then one hardware indirect DMA gathers
  the 64+64 rows (512KB of the 128MB input) and two DMAs write the
  [64, 2048] output slice. Host concatenates the 8 slices.

Fallback (any per-row duplicate id): the original eq/select/reduce sweep
kernel, compiled lazily.
"""
import numpy as np

import concourse.bass as bass
import concourse.tile as tile
from concourse import bacc, library_config, mybir
from concourse import bass_utils

P = 128            # partitions
L = 32768          # B*S tokens
H = 1024           # hidden
NSEG = 512         # segments
NCORES = 8
SEG_PER_CORE = NSEG // NCORES            # 64
TOK_PER_PART = L // P                    # 256 tokens per partition
NELEM = 514        # local_scatter table width (>= 513, even)
F32 = mybir.dt.float32
F16 = mybir.dt.float16
I32 = mybir.dt.int32
I16 = mybir.dt.int16


def build_nc():
    nc = bacc.Bacc("TRN2", target_bir_lowering=False, debug=False)

    x = nc.dram_tensor("x", [L, H], F32, kind="ExternalInput")
    # number_mask int64 raw bytes as int32 (lo, hi) pairs; partition p covers
    # tokens [p*256, (p+1)*256).
    idpairs = nc.dram_tensor("idpairs", [P, TOK_PER_PART, 2], I32, kind="ExternalInput")
    # fp16 consts: cols 0:256 = f+1 (kept base-aligned for the GPSIMD
    # scatter's data operand), col 256 = per-core segment base (64*c)
    enc_in = nc.dram_tensor("encc", [P, TOK_PER_PART + 1], F16, kind="ExternalInput")
    ident_in = nc.dram_tensor("ident", [P, P], F16, kind="ExternalInput")
    # decode bases, both sides on partitions 0:64:
    #   [:, 0, p] (min side): (127 - p) * 256 + 257
    #   [:, 1, p] (max side): 256 * p - 1
    base_in = nc.dram_tensor("base", [SEG_PER_CORE, 2, P], F32, kind="ExternalInput")
    out = nc.dram_tensor("out", [SEG_PER_CORE, 2 * H], F32, kind="ExternalOutput")

    with tile.TileContext(nc) as tc:
        with tc.tile_pool(name="sb", bufs=1) as sb, \
             tc.tile_pool(name="ps", bufs=1, space="PSUM") as ps:

            # hoist the GPSIMD library swap off the critical path: its ucode
            # DMA (~2.4us) then overlaps the input DMAs
            nc.gpsimd.load_library(library_config.local_scatter)

            # critical loads ride the (reliably fast) sync queue; the
            # transpose/decode constants dribble in on the scalar queue
            idp_t = sb.tile([P, TOK_PER_PART, 2], I32)
            nc.sync.dma_start(idp_t[:], idpairs.ap())
            encb = sb.tile([P, TOK_PER_PART + 1], F16)
            nc.sync.dma_start(encb[:], enc_in.ap())
            ident = sb.tile([P, P], F16)
            nc.scalar.dma_start(ident[:], ident_in.ap())
            base_t = sb.tile([SEG_PER_CORE, 2, P], F32)
            nc.scalar.dma_start(base_t[:], base_in.ap())

            # rebase ids to the core's segment range and narrow to int16
            idx16 = sb.tile([P, TOK_PER_PART], I16)
            nc.vector.tensor_tensor(
                out=idx16[:], in0=idp_t[:, :, 0],
                in1=encb[:, TOK_PER_PART:TOK_PER_PART + 1]
                    .broadcast_to([P, TOK_PER_PART]),
                op=mybir.AluOpType.subtract)

            # one scatter builds the whole per-row segment table
            tab = sb.tile([P, NELEM], F16)
            nc.gpsimd.local_scatter(tab[:], encb[:, 0:TOK_PER_PART], idx16[:],
                                    channels=P, num_elems=NELEM,
                                    num_idxs=TOK_PER_PART)

            # one square transpose serves both sides: red_t[s, p] = tab[p, s]
            # for s in 0:64 (cols 64:128 carry other cores' segs, ignored).
            # min side uses base - tab (so no negated copy is needed), max
            # side base + tab; both decodes run on partitions 0:64 so both
            # gather offset tiles are partition-0 based.
            tabd = sb.tile([P, P], F16)
            nc.vector.tensor_copy(tabd[:], tab[:, 0:P])
            red_t = ps.tile([P, P], F16)
            nc.tensor.transpose(out=red_t[:], in_=tabd[:], identity=ident[:])

            # first side: enc = (127-p)*256 + 257 - tab, idx = L - enc
            summ_a = sb.tile([SEG_PER_CORE, P], F32)
            nc.vector.tensor_tensor(out=summ_a[:], in0=base_t[:, 0, :],
                                    in1=red_t[0:SEG_PER_CORE, :],
                                    op=mybir.AluOpType.subtract)
            glob_a = sb.tile([SEG_PER_CORE, P], F32)
            nc.vector.scalar_tensor_tensor(out=glob_a[:],
                                           in0=red_t[0:SEG_PER_CORE, :],
                                           scalar=0.0, in1=summ_a[:],
                                           op0=mybir.AluOpType.not_equal,
                                           op1=mybir.AluOpType.mult)
            enc_a = sb.tile([SEG_PER_CORE, 1], F32)
            nc.vector.tensor_reduce(enc_a[:], glob_a[:],
                                    axis=mybir.AxisListType.X,
                                    op=mybir.AluOpType.max)
            idx_a = sb.tile([SEG_PER_CORE, 1], I32)
            nc.vector.tensor_scalar(idx_a[:], enc_a[:], -1.0, float(L),
                                    op0=mybir.AluOpType.mult,
                                    op1=mybir.AluOpType.add)
            rows_a = sb.tile([SEG_PER_CORE, H], F32)
            nc.gpsimd.indirect_dma_start(
                out=rows_a[:], out_offset=None, in_=x.ap(),
                in_offset=bass.IndirectOffsetOnAxis(ap=idx_a[:, 0:1], axis=0))
            nc.sync.dma_start(out.ap()[:, 0:H], rows_a[:])

            # last side: enc = 256p - 1 + tab = global position, idx = enc
            summ_b = sb.tile([SEG_PER_CORE, P], F32)
            nc.vector.tensor_tensor(out=summ_b[:], in0=base_t[:, 1, :],
                                    in1=red_t[0:SEG_PER_CORE, :],
                                    op=mybir.AluOpType.add)
            glob_b = sb.tile([SEG_PER_CORE, P], F32)
            nc.vector.scalar_tensor_tensor(out=glob_b[:],
                                           in0=red_t[0:SEG_PER_CORE, :],
                                           scalar=0.0, in1=summ_b[:],
                                           op0=mybir.AluOpType.not_equal,
                                           op1=mybir.AluOpType.mult)
            enc_b = sb.tile([SEG_PER_CORE, 1], F32)
            nc.vector.tensor_reduce(enc_b[:], glob_b[:],
                                    axis=mybir.AxisListType.X,
                                    op=mybir.AluOpType.max)
            idx_b = sb.tile([SEG_PER_CORE, 1], I32)
            nc.vector.tensor_copy(idx_b[:], enc_b[:])
            rows_b = sb.tile([SEG_PER_CORE, H], F32)
            nc.gpsimd.indirect_dma_start(
                out=rows_b[:], out_offset=None, in_=x.ap(),
                in_offset=bass.IndirectOffsetOnAxis(ap=idx_b[:, 0:1], axis=0))
            nc.scalar.dma_start(out.ap()[:, H:2 * H], rows_b[:])

    nc.compile()
    return nc


def make_in_maps(input, number_mask):
    x = np.ascontiguousarray(np.asarray(input), dtype=np.float32).reshape(L, H)
    nm = np.ascontiguousarray(np.asarray(number_mask))
    if nm.dtype != np.int64:
        nm = nm.astype(np.int64)
    idpairs = nm.reshape(L).view(np.int32).reshape(P, TOK_PER_PART, 2)
    ident = np.eye(P, dtype=np.float16)
    pcol = np.arange(P, dtype=np.float32)
    base = np.empty((SEG_PER_CORE, 2, P), dtype=np.float32)
    base[:, 0] = (P - 1 - pcol) * TOK_PER_PART + TOK_PER_PART + 1
    base[:, 1] = pcol * TOK_PER_PART - 1
    in_maps = []
    for c in range(NCORES):
        encb = np.empty((P, TOK_PER_PART + 1), dtype=np.float16)
        encb[:, 0:TOK_PER_PART] = np.arange(1, TOK_PER_PART + 1, dtype=np.float16)
        encb[:, TOK_PER_PART] = c * SEG_PER_CORE
        in_maps.append({"x": x, "idpairs": idpairs,
                        "encc": encb, "ident": ident, "base": base})
    return in_maps


# ---------------------------------------------------------------------------
# Fallback: original eq/select/reduce sweep (handles per-row duplicate ids).
# ---------------------------------------------------------------------------

def build_nc_general():
    from concourse.masks import make_identity

    nc = bacc.Bacc("TRN2", target_bir_lowering=False, debug=False)

    x = nc.dram_tensor("x", [L, H], F32, kind="ExternalInput")
    idpairs = nc.dram_tensor("idpairs", [P, TOK_PER_PART, 2], I32, kind="ExternalInput")
    cpack_in = nc.dram_tensor("cpack", [P, 18 * TOK_PER_PART], F16,
                              kind="ExternalInput")
    base_in = nc.dram_tensor("base", [2, SEG_PER_CORE, P], F32, kind="ExternalInput")
    out = nc.dram_tensor("out", [SEG_PER_CORE, 2 * H], F32, kind="ExternalOutput")

    with tile.TileContext(nc) as tc:
        with tc.tile_pool(name="sb", bufs=1) as sb, \
             tc.tile_pool(name="big", bufs=1) as big, \
             tc.tile_pool(name="ps", bufs=1, space="PSUM") as ps:

            idp_t = sb.tile([P, TOK_PER_PART, 2], I32)
            nc.sync.dma_start(idp_t[:], idpairs.ap())
            cpack = sb.tile([P, 18 * TOK_PER_PART], F16)
            nc.scalar.dma_start(cpack[:], cpack_in.ap())
            c8hi_t = cpack[:, 0:8 * TOK_PER_PART].rearrange(
                "p (a t) -> p a t", a=8)
            c8lo_t = cpack[:, 8 * TOK_PER_PART:16 * TOK_PER_PART].rearrange(
                "p (a t) -> p a t", a=8)
            posmin = cpack[:, 16 * TOK_PER_PART:17 * TOK_PER_PART]
            posmax = cpack[:, 17 * TOK_PER_PART:18 * TOK_PER_PART]
            base_t = sb.tile([P, P], F32)
            nc.gpsimd.dma_start(base_t[:], base_in.ap().rearrange("a s p -> (a s) p"))

            hi_i = sb.tile([P, TOK_PER_PART], I32)
            nc.vector.tensor_scalar(hi_i[:], idp_t[:, :, 0], 3, None,
                                    op0=mybir.AluOpType.arith_shift_right)
            lo_i = sb.tile([P, TOK_PER_PART], I32)
            nc.vector.tensor_scalar(lo_i[:], idp_t[:, :, 0], 7, None,
                                    op0=mybir.AluOpType.bitwise_and)
            hi_f = sb.tile([P, TOK_PER_PART], F16)
            nc.vector.tensor_copy(hi_f[:], hi_i[:])
            lo_f = sb.tile([P, TOK_PER_PART], F16)
            nc.vector.tensor_copy(lo_f[:], lo_i[:])

            eq_hi = sb.tile([P, 8, TOK_PER_PART], F16)
            nc.vector.tensor_tensor(
                out=eq_hi[:],
                in0=hi_f[:].unsqueeze(1).broadcast_to([P, 8, TOK_PER_PART]),
                in1=c8hi_t, op=mybir.AluOpType.is_equal)
            eq_lo = sb.tile([P, 8, TOK_PER_PART], F16)
            nc.vector.tensor_tensor(
                out=eq_lo[:],
                in0=lo_f[:].unsqueeze(1).broadcast_to([P, 8, TOK_PER_PART]),
                in1=c8lo_t, op=mybir.AluOpType.is_equal)
            eqlo_min = sb.tile([P, 8, TOK_PER_PART], F16)
            nc.vector.tensor_tensor(
                out=eqlo_min[:], in0=eq_lo[:],
                in1=posmin.unsqueeze(1).broadcast_to([P, 8, TOK_PER_PART]),
                op=mybir.AluOpType.mult)
            eqlo_max = sb.tile([P, 8, TOK_PER_PART], F16)
            nc.vector.tensor_tensor(
                out=eqlo_max[:], in0=eq_lo[:],
                in1=posmax.unsqueeze(1).broadcast_to([P, 8, TOK_PER_PART]),
                op=mybir.AluOpType.mult)

            cand = big.tile([P, 8, 8, TOK_PER_PART], F16)
            nc.vector.tensor_tensor(
                out=cand[:],
                in0=eq_hi[:].unsqueeze(2).broadcast_to([P, 8, 8, TOK_PER_PART]),
                in1=eqlo_min[:].unsqueeze(1).broadcast_to([P, 8, 8, TOK_PER_PART]),
                op=mybir.AluOpType.mult)
            red = sb.tile([P, P], F16)
            c3 = cand[:].rearrange("p a b t -> p (a b) t")
            lv1 = big.tile([P, SEG_PER_CORE, 128], F16, tag="lv1")
            nc.vector.tensor_tensor(out=lv1[:], in0=c3[:, :, 0:128],
                                    in1=c3[:, :, 128:256], op=mybir.AluOpType.max)
            lv2 = sb.tile([P, SEG_PER_CORE, 64], F16, tag="lv2")
            nc.vector.tensor_tensor(out=lv2[:], in0=lv1[:, :, 0:64],
                                    in1=lv1[:, :, 64:128], op=mybir.AluOpType.max)
            lv3 = sb.tile([P, SEG_PER_CORE, 32], F16, tag="lv3")
            nc.vector.tensor_tensor(out=lv3[:], in0=lv2[:, :, 0:32],
                                    in1=lv2[:, :, 32:64], op=mybir.AluOpType.max)
            nc.vector.tensor_reduce(red[:, 0:SEG_PER_CORE], lv3[:],
                                    axis=mybir.AxisListType.X,
                                    op=mybir.AluOpType.max)
            cand2 = big.tile([P, 8, 8, TOK_PER_PART], F16)
            nc.vector.tensor_tensor(
                out=cand2[:],
                in0=eq_hi[:].unsqueeze(2).broadcast_to([P, 8, 8, TOK_PER_PART]),
                in1=eqlo_max[:].unsqueeze(1).broadcast_to([P, 8, 8, TOK_PER_PART]),
                op=mybir.AluOpType.mult)
            c3b = cand2[:].rearrange("p a b t -> p (a b) t")
            lv1b = big.tile([P, SEG_PER_CORE, 128], F16, tag="lv1")
            nc.vector.tensor_tensor(out=lv1b[:], in0=c3b[:, :, 0:128],
                                    in1=c3b[:, :, 128:256], op=mybir.AluOpType.max)
            lv2b = sb.tile([P, SEG_PER_CORE, 64], F16, tag="lv2")
            nc.vector.tensor_tensor(out=lv2b[:], in0=lv1b[:, :, 0:64],
                                    in1=lv1b[:, :, 64:128], op=mybir.AluOpType.max)
            lv3b = sb.tile([P, SEG_PER_CORE, 32], F16, tag="lv3")
            nc.vector.tensor_tensor(out=lv3b[:], in0=lv2b[:, :, 0:32],
                                    in1=lv2b[:, :, 32:64], op=mybir.AluOpType.max)
            nc.vector.tensor_reduce(red[:, SEG_PER_CORE:P], lv3b[:],
                                    axis=mybir.AxisListType.X,
                                    op=mybir.AluOpType.max)

            ident = sb.tile([P, P], F16)
            make_identity(nc, ident[:])
            red_t = ps.tile([P, P], F16)
            nc.tensor.transpose(out=red_t[:], in_=red[:], identity=ident[:])
            mask = sb.tile([P, P], F32)
            nc.vector.tensor_scalar(mask[:], red_t[:], 0.0, None,
                                    op0=mybir.AluOpType.is_gt)
            glob = sb.tile([P, P], F32)
            nc.vector.tensor_tensor(out=glob[:], in0=red_t[:], in1=base_t[:],
                                    op=mybir.AluOpType.add)
            nc.vector.tensor_tensor(out=glob[:], in0=glob[:], in1=mask[:],
                                    op=mybir.AluOpType.mult)
            enc = sb.tile([P, 1], F32)
            nc.vector.tensor_reduce(enc[:], glob[:],
                                    axis=mybir.AxisListType.X,
                                    op=mybir.AluOpType.max)
            idx_f = sb.tile([P, 1], F32)
            nc.vector.tensor_scalar(idx_f[0:SEG_PER_CORE, :], enc[0:SEG_PER_CORE, :],
                                    -1.0, float(L),
                                    op0=mybir.AluOpType.mult,
                                    op1=mybir.AluOpType.add)
            nc.vector.tensor_scalar_add(idx_f[SEG_PER_CORE:P, :],
                                        enc[SEG_PER_CORE:P, :], -1.0)
            idx_i = sb.tile([P, 1], I32)
            nc.vector.tensor_copy(idx_i[:], idx_f[:])
            rows = big.tile([P, H], F32)
            nc.gpsimd.indirect_dma_start(
                out=rows[:], out_offset=None, in_=x.ap(),
                in_offset=bass.IndirectOffsetOnAxis(ap=idx_i[:, 0:1], axis=0))
            nc.gpsimd.dma_start(out.ap()[:, 0:H], rows[0:SEG_PER_CORE, :])
            nc.sync.dma_start(out.ap()[:, H:2 * H], rows[SEG_PER_CORE:P, :])

    nc.compile()
    return nc


def make_in_maps_general(input, number_mask):
    x = np.ascontiguousarray(np.asarray(input), dtype=np.float32).reshape(L, H)
    nm = np.ascontiguousarray(np.asarray(number_mask))
    if nm.dtype != np.int64:
        nm = nm.astype(np.int64)
    idpairs = nm.reshape(L).view(np.int32).reshape(P, TOK_PER_PART, 2)
    c8lo = np.repeat(np.arange(8, dtype=np.float16), TOK_PER_PART)
    f = np.arange(TOK_PER_PART, dtype=np.float16)
    pcol = np.arange(P, dtype=np.float32)
    base = np.empty((2, SEG_PER_CORE, P), dtype=np.float32)
    base[0] = (P - 1 - pcol) * TOK_PER_PART
    base[1] = pcol * TOK_PER_PART
    in_maps = []
    for c in range(NCORES):
        c8hi = np.repeat(np.arange(8, dtype=np.float16) + c * 8, TOK_PER_PART)
        cpack = np.tile(np.concatenate([c8hi, c8lo, TOK_PER_PART - f, f + 1]),
                        (P, 1))
        in_maps.append({"x": x, "idpairs": idpairs, "cpack": cpack,
                        "base": base})
    return in_maps


_NC = None
_NC_GENERAL = None


def _get_nc():
    global _NC
    if _NC is None:
        _NC = build_nc()
    return _NC


def _get_nc_general():
    global _NC_GENERAL
    if _NC_GENERAL is None:
        _NC_GENERAL = build_nc_general()
    return _NC_GENERAL


def _rows_distinct(number_mask):
    ids = np.asarray(number_mask).reshape(P, TOK_PER_PART)
    s = np.sort(ids, axis=1)
    return not np.any(s[:, 1:] == s[:, :-1])


def kernel(input, number_mask, n, concat, **_):
    assert int(n) == NSEG and int(concat) == 1
    if _rows_distinct(number_mask):
        nc = _get_nc()
        in_maps = make_in_maps(input, number_mask)
    else:
        nc = _get_nc_general()
        in_maps = make_in_maps_general(input, number_mask)
    res = bass_utils.run_bass_kernel_spmd(nc, in_maps, core_ids=list(range(NCORES)))
    return np.concatenate([res.results[c]["out"] for c in range(NCORES)], axis=0)


# revision 28
# speedup vs baseline: 2.8100x; 1.0290x over previous
"""Trainium2 Bass kernel for nn_AwareDecoder segment first/last gather.

Problem: input [16, 2048, 1024] f32, number_mask [16, 2048] int64 with ids in
[0, 512]. For each segment id i in [0, 512): find first/last row-major token
position with that id, gather those rows of the flattened input, concat ->
out [512, 2048] f32.

Fast path (8 NeuronCores, segment-sharded - no collectives):
  core c owns segments [64c, 64c+64). Token t = (p, f) with partition
  p = t >> 8 and in-row offset f = t & 255. Within one 256-token row every
  occurring id appears at most once (host-verified; true for the reference's
  arange % 513 mask since 256 < 513), so a single GPSIMD local_scatter builds
  the whole per-row segment table in one shot:

    idx16[p, f] = id[p, f] - 64c   (negative / out-of-range ids are ignored
                                    or land in unused table slots)
    tab[p, v]   = f + 1 where id[p, f] == 64c + v, else 0

  The cross-row combine reuses the encode/transpose/decode trick: cols 0:64
  of a [128, 128] tile hold -tab (first side), cols 64:128 hold +tab (last
  side); PE-transpose, add per-side bases ((127-p)*256 + 257 resp. 256p - 1),
  mask zeros, free-axis max-reduce, then one hardware indirect DMA gathers
  the 64+64 rows (512KB of the 128MB input) and two DMAs write the
  [64, 2048] output slice. Host concatenates the 8 slices.

Fallback (any per-row duplicate id): the original eq/select/reduce sweep
kernel, compiled lazily.
"""
import numpy as np

import concourse.bass as bass
import concourse.tile as tile
from concourse import bacc, library_config, mybir
from concourse import bass_utils

P = 128            # partitions
L = 32768          # B*S tokens
H = 1024           # hidden
NSEG = 512         # segments
NCORES = 8
SEG_PER_CORE = NSEG // NCORES            # 64
TOK_PER_PART = L // P                    # 256 tokens per partition
NELEM = 514        # local_scatter table width (>= 513, even)
F32 = mybir.dt.float32
F16 = mybir.dt.float16
I32 = mybir.dt.int32
I16 = mybir.dt.int16


def build_nc():
    nc = bacc.Bacc("TRN2", target_bir_lowering=False, debug=False)

    x = nc.dram_tensor("x", [L, H], F32, kind="ExternalInput")
    # ids narrowed to int16 host-side (values <= 512); partition p covers
    # tokens [p*256, (p+1)*256).
    idp = nc.dram_tensor("idp", [P, TOK_PER_PART], I16, kind="ExternalInput")
    # two zero offsets used to wake the SWDGE queue early with a tiny gather
    doff_in = nc.dram_tensor("doff", [2, 1], I32, kind="ExternalInput")
    # fp16 consts: cols 0:256 = f+1 (kept base-aligned for the GPSIMD
    # scatter's data operand), col 256 = per-core segment base (64*c)
    enc_in = nc.dram_tensor("encc", [P, TOK_PER_PART + 1], F16, kind="ExternalInput")
    ident_in = nc.dram_tensor("ident", [P, P], F16, kind="ExternalInput")
    # decode bases, both sides on partitions 0:64:
    #   [:, 0, p] (min side): (127 - p) * 256 + 257
    #   [:, 1, p] (max side): 256 * p - 1
    base_in = nc.dram_tensor("base", [SEG_PER_CORE, 2, P], F32, kind="ExternalInput")
    out = nc.dram_tensor("out", [SEG_PER_CORE, 2 * H], F32, kind="ExternalOutput")

    with tile.TileContext(nc) as tc:
        with tc.tile_pool(name="sb", bufs=1) as sb, \
             tc.tile_pool(name="ps", bufs=1, space="PSUM") as ps:

            # hoist the GPSIMD library swap off the critical path: its ucode
            # DMA (~2.4us) then overlaps the input DMAs
            nc.gpsimd.load_library(library_config.local_scatter)

            # critical loads ride the (reliably fast) sync queue; the
            # transpose/decode constants dribble in on the scalar queue
            idp_t = sb.tile([P, TOK_PER_PART], I16)
            nc.sync.dma_start(idp_t[:], idp.ap())
            encb = sb.tile([P, TOK_PER_PART + 1], F16)
            nc.sync.dma_start(encb[:], enc_in.ap())
            doff = sb.tile([2, 1], I32)
            nc.scalar.dma_start(doff[:], doff_in.ap())
            ident = sb.tile([P, P], F16)
            nc.scalar.dma_start(ident[:], ident_in.ap())
            base_t = sb.tile([SEG_PER_CORE, 2, P], F32)
            nc.scalar.dma_start(base_t[:], base_in.ap())

            # wake the SWDGE gather queue long before the real gathers
            scratch = sb.tile([2, H], F32)
            nc.gpsimd.indirect_dma_start(
                out=scratch[:], out_offset=None, in_=x.ap(),
                in_offset=bass.IndirectOffsetOnAxis(ap=doff[:, 0:1], axis=0))

            # rebase ids to the core's segment range
            idx16 = sb.tile([P, TOK_PER_PART], I16)
            nc.vector.tensor_tensor(
                out=idx16[:], in0=idp_t[:],
                in1=encb[:, TOK_PER_PART:TOK_PER_PART + 1]
                    .broadcast_to([P, TOK_PER_PART]),
                op=mybir.AluOpType.subtract)

            # one scatter builds the whole per-row segment table
            tab = sb.tile([P, NELEM], F16)
            nc.gpsimd.local_scatter(tab[:], encb[:, 0:TOK_PER_PART], idx16[:],
                                    channels=P, num_elems=NELEM,
                                    num_idxs=TOK_PER_PART)

            # one square transpose serves both sides: red_t[s, p] = tab[p, s]
            # for s in 0:64 (cols 64:128 carry other cores' segs, ignored).
            # min side uses base - tab (so no negated copy is needed), max
            # side base + tab; both decodes run on partitions 0:64 so both
            # gather offset tiles are partition-0 based.
            tabd = sb.tile([P, P], F16)
            nc.vector.tensor_copy(tabd[:], tab[:, 0:P])
            red_t = ps.tile([P, P], F16)
            nc.tensor.transpose(out=red_t[:], in_=tabd[:], identity=ident[:])

            # first side: enc = (127-p)*256 + 257 - tab, idx = L - enc
            summ_a = sb.tile([SEG_PER_CORE, P], F32)
            nc.vector.tensor_tensor(out=summ_a[:], in0=base_t[:, 0, :],
                                    in1=red_t[0:SEG_PER_CORE, :],
                                    op=mybir.AluOpType.subtract)
            glob_a = sb.tile([SEG_PER_CORE, P], F32)
            nc.vector.scalar_tensor_tensor(out=glob_a[:],
                                           in0=red_t[0:SEG_PER_CORE, :],
                                           scalar=0.0, in1=summ_a[:],
                                           op0=mybir.AluOpType.not_equal,
                                           op1=mybir.AluOpType.mult)
            enc_a = sb.tile([SEG_PER_CORE, 1], F32)
            nc.vector.tensor_reduce(enc_a[:], glob_a[:],
                                    axis=mybir.AxisListType.X,
                                    op=mybir.AluOpType.max)
            idx_a = sb.tile([SEG_PER_CORE, 1], I32)
            nc.vector.tensor_scalar(idx_a[:], enc_a[:], -1.0, float(L),
                                    op0=mybir.AluOpType.mult,
                                    op1=mybir.AluOpType.add)
            rows_a = sb.tile([SEG_PER_CORE, H], F32)
            nc.gpsimd.indirect_dma_start(
                out=rows_a[:], out_offset=None, in_=x.ap(),
                in_offset=bass.IndirectOffsetOnAxis(ap=idx_a[:, 0:1], axis=0))
            nc.sync.dma_start(out.ap()[:, 0:H], rows_a[:])

            # last side: enc = 256p - 1 + tab = global position, idx = enc
            summ_b = sb.tile([SEG_PER_CORE, P], F32)
            nc.vector.tensor_tensor(out=summ_b[:], in0=base_t[:, 1, :],
                                    in1=red_t[0:SEG_PER_CORE, :],
                                    op=mybir.AluOpType.add)
            glob_b = sb.tile([SEG_PER_CORE, P], F32)
            nc.vector.scalar_tensor_tensor(out=glob_b[:],
                                           in0=red_t[0:SEG_PER_CORE, :],
                                           scalar=0.0, in1=summ_b[:],
                                           op0=mybir.AluOpType.not_equal,
                                           op1=mybir.AluOpType.mult)
            enc_b = sb.tile([SEG_PER_CORE, 1], F32)
            nc.vector.tensor_reduce(enc_b[:], glob_b[:],
                                    axis=mybir.AxisListType.X,
                                    op=mybir.AluOpType.max)
            idx_b = sb.tile([SEG_PER_CORE, 1], I32)
            nc.vector.tensor_copy(idx_b[:], enc_b[:])
            rows_b = sb.tile([SEG_PER_CORE, H], F32)
            nc.gpsimd.indirect_dma_start(
                out=rows_b[:], out_offset=None, in_=x.ap(),
                in_offset=bass.IndirectOffsetOnAxis(ap=idx_b[:, 0:1], axis=0))
            nc.scalar.dma_start(out.ap()[:, H:2 * H], rows_b[:])

    nc.compile()
    return nc


def make_in_maps(input, number_mask):
    x = np.ascontiguousarray(np.asarray(input), dtype=np.float32).reshape(L, H)
    nm = np.ascontiguousarray(np.asarray(number_mask))
    if nm.dtype != np.int64:
        nm = nm.astype(np.int64)
    idp16 = nm.reshape(P, TOK_PER_PART).astype(np.int16)
    doff = np.zeros((2, 1), dtype=np.int32)
    ident = np.eye(P, dtype=np.float16)
    pcol = np.arange(P, dtype=np.float32)
    base = np.empty((SEG_PER_CORE, 2, P), dtype=np.float32)
    base[:, 0] = (P - 1 - pcol) * TOK_PER_PART + TOK_PER_PART + 1
    base[:, 1] = pcol * TOK_PER_PART - 1
    in_maps = []
    for c in range(NCORES):
        encb = np.empty((P, TOK_PER_PART + 1), dtype=np.float16)
        encb[:, 0:TOK_PER_PART] = np.arange(1, TOK_PER_PART + 1, dtype=np.float16)
        encb[:, TOK_PER_PART] = c * SEG_PER_CORE
        in_maps.append({"x": x, "idp": idp16, "doff": doff,
                        "encc": encb, "ident": ident, "base": base})
    return in_maps


# ---------------------------------------------------------------------------
# Fallback: original eq/select/reduce sweep (handles per-row duplicate ids).
# ---------------------------------------------------------------------------

def build_nc_general():
    from concourse.masks import make_identity

    nc = bacc.Bacc("TRN2", target_bir_lowering=False, debug=False)

    x = nc.dram_tensor("x", [L, H], F32, kind="ExternalInput")
    idpairs = nc.dram_tensor("idpairs", [P, TOK_PER_PART, 2], I32, kind="ExternalInput")
    cpack_in = nc.dram_tensor("cpack", [P, 18 * TOK_PER_PART], F16,
                              kind="ExternalInput")
    base_in = nc.dram_tensor("base", [2, SEG_PER_CORE, P], F32, kind="ExternalInput")
    out = nc.dram_tensor("out", [SEG_PER_CORE, 2 * H], F32, kind="ExternalOutput")

    with tile.TileContext(nc) as tc:
        with tc.tile_pool(name="sb", bufs=1) as sb, \
             tc.tile_pool(name="big", bufs=1) as big, \
             tc.tile_pool(name="ps", bufs=1, space="PSUM") as ps:

            idp_t = sb.tile([P, TOK_PER_PART, 2], I32)
            nc.sync.dma_start(idp_t[:], idpairs.ap())
            cpack = sb.tile([P, 18 * TOK_PER_PART], F16)
            nc.scalar.dma_start(cpack[:], cpack_in.ap())
            c8hi_t = cpack[:, 0:8 * TOK_PER_PART].rearrange(
                "p (a t) -> p a t", a=8)
            c8lo_t = cpack[:, 8 * TOK_PER_PART:16 * TOK_PER_PART].rearrange(
                "p (a t) -> p a t", a=8)
            posmin = cpack[:, 16 * TOK_PER_PART:17 * TOK_PER_PART]
            posmax = cpack[:, 17 * TOK_PER_PART:18 * TOK_PER_PART]
            base_t = sb.tile([P, P], F32)
            nc.gpsimd.dma_start(base_t[:], base_in.ap().rearrange("a s p -> (a s) p"))

            hi_i = sb.tile([P, TOK_PER_PART], I32)
            nc.vector.tensor_scalar(hi_i[:], idp_t[:, :, 0], 3, None,
                                    op0=mybir.AluOpType.arith_shift_right)
            lo_i = sb.tile([P, TOK_PER_PART], I32)
            nc.vector.tensor_scalar(lo_i[:], idp_t[:, :, 0], 7, None,
                                    op0=mybir.AluOpType.bitwise_and)
            hi_f = sb.tile([P, TOK_PER_PART], F16)
            nc.vector.tensor_copy(hi_f[:], hi_i[:])
            lo_f = sb.tile([P, TOK_PER_PART], F16)
            nc.vector.tensor_copy(lo_f[:], lo_i[:])

            eq_hi = sb.tile([P, 8, TOK_PER_PART], F16)
            nc.vector.tensor_tensor(
                out=eq_hi[:],
                in0=hi_f[:].unsqueeze(1).broadcast_to([P, 8, TOK_PER_PART]),
                in1=c8hi_t, op=mybir.AluOpType.is_equal)
            eq_lo = sb.tile([P, 8, TOK_PER_PART], F16)
            nc.vector.tensor_tensor(
                out=eq_lo[:],
                in0=lo_f[:].unsqueeze(1).broadcast_to([P, 8, TOK_PER_PART]),
                in1=c8lo_t, op=mybir.AluOpType.is_equal)
            eqlo_min = sb.tile([P, 8, TOK_PER_PART], F16)
            nc.vector.tensor_tensor(
                out=eqlo_min[:], in0=eq_lo[:],
                in1=posmin.unsqueeze(1).broadcast_to([P, 8, TOK_PER_PART]),
                op=mybir.AluOpType.mult)
            eqlo_max = sb.tile([P, 8, TOK_PER_PART], F16)
            nc.vector.tensor_tensor(
                out=eqlo_max[:], in0=eq_lo[:],
                in1=posmax.unsqueeze(1).broadcast_to([P, 8, TOK_PER_PART]),
                op=mybir.AluOpType.mult)

            cand = big.tile([P, 8, 8, TOK_PER_PART], F16)
            nc.vector.tensor_tensor(
                out=cand[:],
                in0=eq_hi[:].unsqueeze(2).broadcast_to([P, 8, 8, TOK_PER_PART]),
                in1=eqlo_min[:].unsqueeze(1).broadcast_to([P, 8, 8, TOK_PER_PART]),
                op=mybir.AluOpType.mult)
            red = sb.tile([P, P], F16)
            c3 = cand[:].rearrange("p a b t -> p (a b) t")
            lv1 = big.tile([P, SEG_PER_CORE, 128], F16, tag="lv1")
            nc.vector.tensor_tensor(out=lv1[:], in0=c3[:, :, 0:128],
                                    in1=c3[:, :, 128:256], op=mybir.AluOpType.max)
            lv2 = sb.tile([P, SEG_PER_CORE, 64], F16, tag="lv2")
            nc.vector.tensor_tensor(out=lv2[:], in0=lv1[:, :, 0:64],
                                    in1=lv1[:, :, 64:128], op=mybir.AluOpType.max)
            lv3 = sb.tile([P, SEG_PER_CORE, 32], F16, tag="lv3")
            nc.vector.tensor_tensor(out=lv3[:], in0=lv2[:, :, 0:32],
                                    in1=lv2[:, :, 32:64], op=mybir.AluOpType.max)
            nc.vector.tensor_reduce(red[:, 0:SEG_PER_CORE], lv3[:],
                                    axis=mybir.AxisListType.X,
                                    op=mybir.AluOpType.max)
            cand2 = big.tile([P, 8, 8, TOK_PER_PART], F16)
            nc.vector.tensor_tensor(
                out=cand2[:],
                in0=eq_hi[:].unsqueeze(2).broadcast_to([P, 8, 8, TOK_PER_PART]),
                in1=eqlo_max[:].unsqueeze(1).broadcast_to([P, 8, 8, TOK_PER_PART]),
                op=mybir.AluOpType.mult)
            c3b = cand2[:].rearrange("p a b t -> p (a b) t")
            lv1b = big.tile([P, SEG_PER_CORE, 128], F16, tag="lv1")
            nc.vector.tensor_tensor(out=lv1b[:], in0=c3b[:, :, 0:128],
                                    in1=c3b[:, :, 128:256], op=mybir.AluOpType.max)
            lv2b = sb.tile([P, SEG_PER_CORE, 64], F16, tag="lv2")
            nc.vector.tensor_tensor(out=lv2b[:], in0=lv1b[:, :, 0:64],
                                    in1=lv1b[:, :, 64:128], op=mybir.AluOpType.max)
            lv3b = sb.tile([P, SEG_PER_CORE, 32], F16, tag="lv3")
            nc.vector.tensor_tensor(out=lv3b[:], in0=lv2b[:, :, 0:32],
                                    in1=lv2b[:, :, 32:64], op=mybir.AluOpType.max)
            nc.vector.tensor_reduce(red[:, SEG_PER_CORE:P], lv3b[:],
                                    axis=mybir.AxisListType.X,
                                    op=mybir.AluOpType.max)

            ident = sb.tile([P, P], F16)
            make_identity(nc, ident[:])
            red_t = ps.tile([P, P], F16)
            nc.tensor.transpose(out=red_t[:], in_=red[:], identity=ident[:])
            mask = sb.tile([P, P], F32)
            nc.vector.tensor_scalar(mask[:], red_t[:], 0.0, None,
                                    op0=mybir.AluOpType.is_gt)
            glob = sb.tile([P, P], F32)
            nc.vector.tensor_tensor(out=glob[:], in0=red_t[:], in1=base_t[:],
                                    op=mybir.AluOpType.add)
            nc.vector.tensor_tensor(out=glob[:], in0=glob[:], in1=mask[:],
                                    op=mybir.AluOpType.mult)
            enc = sb.tile([P, 1], F32)
            nc.vector.tensor_reduce(enc[:], glob[:],
                                    axis=mybir.AxisListType.X,
                                    op=mybir.AluOpType.max)
            idx_f = sb.tile([P, 1], F32)
            nc.vector.tensor_scalar(idx_f[0:SEG_PER_CORE, :], enc[0:SEG_PER_CORE, :],
                                    -1.0, float(L),
                                    op0=mybir.AluOpType.mult,
                                    op1=mybir.AluOpType.add)
            nc.vector.tensor_scalar_add(idx_f[SEG_PER_CORE:P, :],
                                        enc[SEG_PER_CORE:P, :], -1.0)
            idx_i = sb.tile([P, 1], I32)
            nc.vector.tensor_copy(idx_i[:], idx_f[:])
            rows = big.tile([P, H], F32)
            nc.gpsimd.indirect_dma_start(
                out=rows[:], out_offset=None, in_=x.ap(),
                in_offset=bass.IndirectOffsetOnAxis(ap=idx_i[:, 0:1], axis=0))
            nc.gpsimd.dma_start(out.ap()[:, 0:H], rows[0:SEG_PER_CORE, :])
            nc.sync.dma_start(out.ap()[:, H:2 * H], rows[SEG_PER_CORE:P, :])

    nc.compile()
    return nc


def make_in_maps_general(input, number_mask):
    x = np.ascontiguousarray(np.asarray(input), dtype=np.float32).reshape(L, H)
    nm = np.ascontiguousarray(np.asarray(number_mask))
    if nm.dtype != np.int64:
        nm = nm.astype(np.int64)
    idpairs = nm.reshape(L).view(np.int32).reshape(P, TOK_PER_PART, 2)
    c8lo = np.repeat(np.arange(8, dtype=np.float16), TOK_PER_PART)
    f = np.arange(TOK_PER_PART, dtype=np.float16)
    pcol = np.arange(P, dtype=np.float32)
    base = np.empty((2, SEG_PER_CORE, P), dtype=np.float32)
    base[0] = (P - 1 - pcol) * TOK_PER_PART
    base[1] = pcol * TOK_PER_PART
    in_maps = []
    for c in range(NCORES):
        c8hi = np.repeat(np.arange(8, dtype=np.float16) + c * 8, TOK_PER_PART)
        cpack = np.tile(np.concatenate([c8hi, c8lo, TOK_PER_PART - f, f + 1]),
                        (P, 1))
        in_maps.append({"x": x, "idpairs": idpairs, "cpack": cpack,
                        "base": base})
    return in_maps


_NC = None
_NC_GENERAL = None


def _get_nc():
    global _NC
    if _NC is None:
        _NC = build_nc()
    return _NC


def _get_nc_general():
    global _NC_GENERAL
    if _NC_GENERAL is None:
        _NC_GENERAL = build_nc_general()
    return _NC_GENERAL


def _rows_distinct(number_mask):
    ids = np.asarray(number_mask).reshape(P, TOK_PER_PART)
    s = np.sort(ids, axis=1)
    return not np.any(s[:, 1:] == s[:, :-1])


def kernel(input, number_mask, n, concat, **_):
    assert int(n) == NSEG and int(concat) == 1
    if _rows_distinct(number_mask):
        nc = _get_nc()
        in_maps = make_in_maps(input, number_mask)
    else:
        nc = _get_nc_general()
        in_maps = make_in_maps_general(input, number_mask)
    res = bass_utils.run_bass_kernel_spmd(nc, in_maps, core_ids=list(range(NCORES)))
    return np.concatenate([res.results[c]["out"] for c in range(NCORES)], axis=0)


# revision 33
# speedup vs baseline: 2.8772x; 1.0239x over previous
"""Trainium2 Bass kernel for nn_AwareDecoder segment first/last gather.

Problem: input [16, 2048, 1024] f32, number_mask [16, 2048] int64 with ids in
[0, 512]. For each segment id i in [0, 512): find first/last row-major token
position with that id, gather those rows of the flattened input, concat ->
out [512, 2048] f32.

Fast path (8 NeuronCores, segment-sharded - no collectives):
  core c owns segments [64c, 64c+64). Token t = (p, f) with partition
  p = t >> 8 and in-row offset f = t & 255. Within one 256-token row every
  occurring id appears at most once (host-verified; true for the reference's
  arange % 513 mask since 256 < 513), so a single GPSIMD local_scatter builds
  the whole per-row segment table in one shot:

    idx16[p, f] = id[p, f] - 64c   (negative / out-of-range ids are ignored
                                    or land in unused table slots)
    tab[p, v]   = f + 1 where id[p, f] == 64c + v, else 0

  The cross-row combine reuses the encode/transpose/decode trick: cols 0:64
  of a [128, 128] tile hold -tab (first side), cols 64:128 hold +tab (last
  side); PE-transpose, add per-side bases ((127-p)*256 + 257 resp. 256p - 1),
  mask zeros, free-axis max-reduce, then one hardware indirect DMA gathers
  the 64+64 rows (512KB of the 128MB input) and two DMAs write the
  [64, 2048] output slice. Host concatenates the 8 slices.

Fallback (any per-row duplicate id): the original eq/select/reduce sweep
kernel, compiled lazily.
"""
import numpy as np

import concourse.bass as bass
import concourse.tile as tile
from concourse import bacc, library_config, mybir
from concourse import bass_utils

P = 128            # partitions
L = 32768          # B*S tokens
H = 1024           # hidden
NSEG = 512         # segments
NCORES = 8
SEG_PER_CORE = NSEG // NCORES            # 64
TOK_PER_PART = L // P                    # 256 tokens per partition
NELEM = 514        # local_scatter table width (>= 513, even)
F32 = mybir.dt.float32
F16 = mybir.dt.float16
I32 = mybir.dt.int32
I16 = mybir.dt.int16


def build_nc():
    nc = bacc.Bacc("TRN2", target_bir_lowering=False, debug=False)

    x = nc.dram_tensor("x", [L, H], F32, kind="ExternalInput")
    # ids narrowed to int16 host-side (values <= 512); partition p covers
    # tokens [p*256, (p+1)*256).
    idp = nc.dram_tensor("idp", [P, TOK_PER_PART], I16, kind="ExternalInput")
    # fp16 consts: cols 0:256 = f+1 (kept base-aligned for the GPSIMD
    # scatter's data operand), col 256 = per-core segment base (64*c)
    enc_in = nc.dram_tensor("encc", [P, TOK_PER_PART + 1], F16, kind="ExternalInput")
    ident_in = nc.dram_tensor("ident", [P, P], F16, kind="ExternalInput")
    # decode bases, both sides on partitions 0:64:
    #   [:, 0, p] (min side): (127 - p) * 256 + 257
    #   [:, 1, p] (max side): 256 * p - 1
    base_in = nc.dram_tensor("base", [SEG_PER_CORE, 2, P], F32, kind="ExternalInput")
    out = nc.dram_tensor("out", [SEG_PER_CORE, 2 * H], F32, kind="ExternalOutput")

    with tile.TileContext(nc) as tc:
        with tc.tile_pool(name="sb", bufs=1) as sb, \
             tc.tile_pool(name="ps", bufs=1, space="PSUM") as ps:

            # hoist the GPSIMD library swap off the critical path: its ucode
            # DMA (~2.4us) then overlaps the input DMAs
            nc.gpsimd.load_library(library_config.local_scatter)

            # critical loads ride the (reliably fast) sync queue; the
            # transpose/decode constants dribble in on the scalar queue
            idp_t = sb.tile([P, TOK_PER_PART], I16)
            nc.sync.dma_start(idp_t[:], idp.ap())
            encb = sb.tile([P, TOK_PER_PART + 1], F16)
            nc.scalar.dma_start(encb[:], enc_in.ap())
            ident = sb.tile([P, P], F16)
            nc.scalar.dma_start(ident[:], ident_in.ap())
            base_t = sb.tile([SEG_PER_CORE, 2, P], F32)
            nc.scalar.dma_start(base_t[:], base_in.ap())

            # rebase ids to the core's segment range
            idx16 = sb.tile([P, TOK_PER_PART], I16)
            nc.vector.tensor_tensor(
                out=idx16[:], in0=idp_t[:],
                in1=encb[:, TOK_PER_PART:TOK_PER_PART + 1]
                    .broadcast_to([P, TOK_PER_PART]),
                op=mybir.AluOpType.subtract)

            # one scatter builds the whole per-row segment table
            tab = sb.tile([P, NELEM], F16)
            nc.gpsimd.local_scatter(tab[:], encb[:, 0:TOK_PER_PART], idx16[:],
                                    channels=P, num_elems=NELEM,
                                    num_idxs=TOK_PER_PART)

            # one square transpose serves both sides: red_t[s, p] = tab[p, s]
            # for s in 0:64 (cols 64:128 carry other cores' segs, ignored).
            # min side uses base - tab (so no negated copy is needed), max
            # side base + tab; both decodes run on partitions 0:64 so both
            # gather offset tiles are partition-0 based.
            red_t = ps.tile([P, P], F16)
            nc.tensor.transpose(out=red_t[:], in_=tab[:, 0:P], identity=ident[:])

            # first side: enc = (127-p)*256 + 257 - tab, idx = L - enc
            summ_a = sb.tile([SEG_PER_CORE, P], F32)
            nc.vector.tensor_tensor(out=summ_a[:], in0=base_t[:, 0, :],
                                    in1=red_t[0:SEG_PER_CORE, :],
                                    op=mybir.AluOpType.subtract)
            glob_a = sb.tile([SEG_PER_CORE, P], F32)
            nc.vector.scalar_tensor_tensor(out=glob_a[:],
                                           in0=red_t[0:SEG_PER_CORE, :],
                                           scalar=0.0, in1=summ_a[:],
                                           op0=mybir.AluOpType.not_equal,
                                           op1=mybir.AluOpType.mult)
            enc_a = sb.tile([SEG_PER_CORE, 1], F32)
            nc.vector.tensor_reduce(enc_a[:], glob_a[:],
                                    axis=mybir.AxisListType.X,
                                    op=mybir.AluOpType.max)
            idx_a = sb.tile([SEG_PER_CORE, 1], I32)
            nc.vector.tensor_scalar(idx_a[:], enc_a[:], -1.0, float(L),
                                    op0=mybir.AluOpType.mult,
                                    op1=mybir.AluOpType.add)
            rows_a = sb.tile([SEG_PER_CORE, H], F32)
            nc.gpsimd.indirect_dma_start(
                out=rows_a[:], out_offset=None, in_=x.ap(),
                in_offset=bass.IndirectOffsetOnAxis(ap=idx_a[:, 0:1], axis=0))
            nc.sync.dma_start(out.ap()[:, 0:H], rows_a[:])

            # last side: enc = 256p - 1 + tab = global position, idx = enc
            summ_b = sb.tile([SEG_PER_CORE, P], F32)
            nc.vector.tensor_tensor(out=summ_b[:], in0=base_t[:, 1, :],
                                    in1=red_t[0:SEG_PER_CORE, :],
                                    op=mybir.AluOpType.add)
            glob_b = sb.tile([SEG_PER_CORE, P], F32)
            nc.vector.scalar_tensor_tensor(out=glob_b[:],
                                           in0=red_t[0:SEG_PER_CORE, :],
                                           scalar=0.0, in1=summ_b[:],
                                           op0=mybir.AluOpType.not_equal,
                                           op1=mybir.AluOpType.mult)
            enc_b = sb.tile([SEG_PER_CORE, 1], F32)
            nc.vector.tensor_reduce(enc_b[:], glob_b[:],
                                    axis=mybir.AxisListType.X,
                                    op=mybir.AluOpType.max)
            idx_b = sb.tile([SEG_PER_CORE, 1], I32)
            nc.vector.tensor_copy(idx_b[:], enc_b[:])
            rows_b = sb.tile([SEG_PER_CORE, H], F32)
            nc.gpsimd.indirect_dma_start(
                out=rows_b[:], out_offset=None, in_=x.ap(),
                in_offset=bass.IndirectOffsetOnAxis(ap=idx_b[:, 0:1], axis=0))
            nc.scalar.dma_start(out.ap()[:, H:2 * H], rows_b[:])

    nc.compile()
    return nc


def make_in_maps(input, number_mask):
    x = np.ascontiguousarray(np.asarray(input), dtype=np.float32).reshape(L, H)
    nm = np.ascontiguousarray(np.asarray(number_mask))
    if nm.dtype != np.int64:
        nm = nm.astype(np.int64)
    idp16 = nm.reshape(P, TOK_PER_PART).astype(np.int16)
    ident = np.eye(P, dtype=np.float16)
    pcol = np.arange(P, dtype=np.float32)
    base = np.empty((SEG_PER_CORE, 2, P), dtype=np.float32)
    base[:, 0] = (P - 1 - pcol) * TOK_PER_PART + TOK_PER_PART + 1
    base[:, 1] = pcol * TOK_PER_PART - 1
    in_maps = []
    for c in range(NCORES):
        encb = np.empty((P, TOK_PER_PART + 1), dtype=np.float16)
        encb[:, 0:TOK_PER_PART] = np.arange(1, TOK_PER_PART + 1, dtype=np.float16)
        encb[:, TOK_PER_PART] = c * SEG_PER_CORE
        in_maps.append({"x": x, "idp": idp16,
                        "encc": encb, "ident": ident, "base": base})
    return in_maps


# ---------------------------------------------------------------------------
# Fallback: original eq/select/reduce sweep (handles per-row duplicate ids).
# ---------------------------------------------------------------------------

def build_nc_general():
    from concourse.masks import make_identity

    nc = bacc.Bacc("TRN2", target_bir_lowering=False, debug=False)

    x = nc.dram_tensor("x", [L, H], F32, kind="ExternalInput")
    idpairs = nc.dram_tensor("idpairs", [P, TOK_PER_PART, 2], I32, kind="ExternalInput")
    cpack_in = nc.dram_tensor("cpack", [P, 18 * TOK_PER_PART], F16,
                              kind="ExternalInput")
    base_in = nc.dram_tensor("base", [2, SEG_PER_CORE, P], F32, kind="ExternalInput")
    out = nc.dram_tensor("out", [SEG_PER_CORE, 2 * H], F32, kind="ExternalOutput")

    with tile.TileContext(nc) as tc:
        with tc.tile_pool(name="sb", bufs=1) as sb, \
             tc.tile_pool(name="big", bufs=1) as big, \
             tc.tile_pool(name="ps", bufs=1, space="PSUM") as ps:

            idp_t = sb.tile([P, TOK_PER_PART, 2], I32)
            nc.sync.dma_start(idp_t[:], idpairs.ap())
            cpack = sb.tile([P, 18 * TOK_PER_PART], F16)
            nc.scalar.dma_start(cpack[:], cpack_in.ap())
            c8hi_t = cpack[:, 0:8 * TOK_PER_PART].rearrange(
                "p (a t) -> p a t", a=8)
            c8lo_t = cpack[:, 8 * TOK_PER_PART:16 * TOK_PER_PART].rearrange(
                "p (a t) -> p a t", a=8)
            posmin = cpack[:, 16 * TOK_PER_PART:17 * TOK_PER_PART]
            posmax = cpack[:, 17 * TOK_PER_PART:18 * TOK_PER_PART]
            base_t = sb.tile([P, P], F32)
            nc.gpsimd.dma_start(base_t[:], base_in.ap().rearrange("a s p -> (a s) p"))

            hi_i = sb.tile([P, TOK_PER_PART], I32)
            nc.vector.tensor_scalar(hi_i[:], idp_t[:, :, 0], 3, None,
                                    op0=mybir.AluOpType.arith_shift_right)
            lo_i = sb.tile([P, TOK_PER_PART], I32)
            nc.vector.tensor_scalar(lo_i[:], idp_t[:, :, 0], 7, None,
                                    op0=mybir.AluOpType.bitwise_and)
            hi_f = sb.tile([P, TOK_PER_PART], F16)
            nc.vector.tensor_copy(hi_f[:], hi_i[:])
            lo_f = sb.tile([P, TOK_PER_PART], F16)
            nc.vector.tensor_copy(lo_f[:], lo_i[:])

            eq_hi = sb.tile([P, 8, TOK_PER_PART], F16)
            nc.vector.tensor_tensor(
                out=eq_hi[:],
                in0=hi_f[:].unsqueeze(1).broadcast_to([P, 8, TOK_PER_PART]),
                in1=c8hi_t, op=mybir.AluOpType.is_equal)
            eq_lo = sb.tile([P, 8, TOK_PER_PART], F16)
            nc.vector.tensor_tensor(
                out=eq_lo[:],
                in0=lo_f[:].unsqueeze(1).broadcast_to([P, 8, TOK_PER_PART]),
                in1=c8lo_t, op=mybir.AluOpType.is_equal)
            eqlo_min = sb.tile([P, 8, TOK_PER_PART], F16)
            nc.vector.tensor_tensor(
                out=eqlo_min[:], in0=eq_lo[:],
                in1=posmin.unsqueeze(1).broadcast_to([P, 8, TOK_PER_PART]),
                op=mybir.AluOpType.mult)
            eqlo_max = sb.tile([P, 8, TOK_PER_PART], F16)
            nc.vector.tensor_tensor(
                out=eqlo_max[:], in0=eq_lo[:],
                in1=posmax.unsqueeze(1).broadcast_to([P, 8, TOK_PER_PART]),
                op=mybir.AluOpType.mult)

            cand = big.tile([P, 8, 8, TOK_PER_PART], F16)
            nc.vector.tensor_tensor(
                out=cand[:],
                in0=eq_hi[:].unsqueeze(2).broadcast_to([P, 8, 8, TOK_PER_PART]),
                in1=eqlo_min[:].unsqueeze(1).broadcast_to([P, 8, 8, TOK_PER_PART]),
                op=mybir.AluOpType.mult)
            red = sb.tile([P, P], F16)
            c3 = cand[:].rearrange("p a b t -> p (a b) t")
            lv1 = big.tile([P, SEG_PER_CORE, 128], F16, tag="lv1")
            nc.vector.tensor_tensor(out=lv1[:], in0=c3[:, :, 0:128],
                                    in1=c3[:, :, 128:256], op=mybir.AluOpType.max)
            lv2 = sb.tile([P, SEG_PER_CORE, 64], F16, tag="lv2")
            nc.vector.tensor_tensor(out=lv2[:], in0=lv1[:, :, 0:64],
                                    in1=lv1[:, :, 64:128], op=mybir.AluOpType.max)
            lv3 = sb.tile([P, SEG_PER_CORE, 32], F16, tag="lv3")
            nc.vector.tensor_tensor(out=lv3[:], in0=lv2[:, :, 0:32],
                                    in1=lv2[:, :, 32:64], op=mybir.AluOpType.max)
            nc.vector.tensor_reduce(red[:, 0:SEG_PER_CORE], lv3[:],
                                    axis=mybir.AxisListType.X,
                                    op=mybir.AluOpType.max)
            cand2 = big.tile([P, 8, 8, TOK_PER_PART], F16)
            nc.vector.tensor_tensor(
                out=cand2[:],
                in0=eq_hi[:].unsqueeze(2).broadcast_to([P, 8, 8, TOK_PER_PART]),
                in1=eqlo_max[:].unsqueeze(1).broadcast_to([P, 8, 8, TOK_PER_PART]),
                op=mybir.AluOpType.mult)
            c3b = cand2[:].rearrange("p a b t -> p (a b) t")
            lv1b = big.tile([P, SEG_PER_CORE, 128], F16, tag="lv1")
            nc.vector.tensor_tensor(out=lv1b[:], in0=c3b[:, :, 0:128],
                                    in1=c3b[:, :, 128:256], op=mybir.AluOpType.max)
            lv2b = sb.tile([P, SEG_PER_CORE, 64], F16, tag="lv2")
            nc.vector.tensor_tensor(out=lv2b[:], in0=lv1b[:, :, 0:64],
                                    in1=lv1b[:, :, 64:128], op=mybir.AluOpType.max)
            lv3b = sb.tile([P, SEG_PER_CORE, 32], F16, tag="lv3")
            nc.vector.tensor_tensor(out=lv3b[:], in0=lv2b[:, :, 0:32],
                                    in1=lv2b[:, :, 32:64], op=mybir.AluOpType.max)
            nc.vector.tensor_reduce(red[:, SEG_PER_CORE:P], lv3b[:],
                                    axis=mybir.AxisListType.X,
                                    op=mybir.AluOpType.max)

            ident = sb.tile([P, P], F16)
            make_identity(nc, ident[:])
            red_t = ps.tile([P, P], F16)
            nc.tensor.transpose(out=red_t[:], in_=red[:], identity=ident[:])
            mask = sb.tile([P, P], F32)
            nc.vector.tensor_scalar(mask[:], red_t[:], 0.0, None,
                                    op0=mybir.AluOpType.is_gt)
            glob = sb.tile([P, P], F32)
            nc.vector.tensor_tensor(out=glob[:], in0=red_t[:], in1=base_t[:],
                                    op=mybir.AluOpType.add)
            nc.vector.tensor_tensor(out=glob[:], in0=glob[:], in1=mask[:],
                                    op=mybir.AluOpType.mult)
            enc = sb.tile([P, 1], F32)
            nc.vector.tensor_reduce(enc[:], glob[:],
                                    axis=mybir.AxisListType.X,
                                    op=mybir.AluOpType.max)
            idx_f = sb.tile([P, 1], F32)
            nc.vector.tensor_scalar(idx_f[0:SEG_PER_CORE, :], enc[0:SEG_PER_CORE, :],
                                    -1.0, float(L),
                                    op0=mybir.AluOpType.mult,
                                    op1=mybir.AluOpType.add)
            nc.vector.tensor_scalar_add(idx_f[SEG_PER_CORE:P, :],
                                        enc[SEG_PER_CORE:P, :], -1.0)
            idx_i = sb.tile([P, 1], I32)
            nc.vector.tensor_copy(idx_i[:], idx_f[:])
            rows = big.tile([P, H], F32)
            nc.gpsimd.indirect_dma_start(
                out=rows[:], out_offset=None, in_=x.ap(),
                in_offset=bass.IndirectOffsetOnAxis(ap=idx_i[:, 0:1], axis=0))
            nc.gpsimd.dma_start(out.ap()[:, 0:H], rows[0:SEG_PER_CORE, :])
            nc.sync.dma_start(out.ap()[:, H:2 * H], rows[SEG_PER_CORE:P, :])

    nc.compile()
    return nc


def make_in_maps_general(input, number_mask):
    x = np.ascontiguousarray(np.asarray(input), dtype=np.float32).reshape(L, H)
    nm = np.ascontiguousarray(np.asarray(number_mask))
    if nm.dtype != np.int64:
        nm = nm.astype(np.int64)
    idpairs = nm.reshape(L).view(np.int32).reshape(P, TOK_PER_PART, 2)
    c8lo = np.repeat(np.arange(8, dtype=np.float16), TOK_PER_PART)
    f = np.arange(TOK_PER_PART, dtype=np.float16)
    pcol = np.arange(P, dtype=np.float32)
    base = np.empty((2, SEG_PER_CORE, P), dtype=np.float32)
    base[0] = (P - 1 - pcol) * TOK_PER_PART
    base[1] = pcol * TOK_PER_PART
    in_maps = []
    for c in range(NCORES):
        c8hi = np.repeat(np.arange(8, dtype=np.float16) + c * 8, TOK_PER_PART)
        cpack = np.tile(np.concatenate([c8hi, c8lo, TOK_PER_PART - f, f + 1]),
                        (P, 1))
        in_maps.append({"x": x, "idpairs": idpairs, "cpack": cpack,
                        "base": base})
    return in_maps


_NC = None
_NC_GENERAL = None


def _get_nc():
    global _NC
    if _NC is None:
        _NC = build_nc()
    return _NC


def _get_nc_general():
    global _NC_GENERAL
    if _NC_GENERAL is None:
        _NC_GENERAL = build_nc_general()
    return _NC_GENERAL


def _rows_distinct(number_mask):
    ids = np.asarray(number_mask).reshape(P, TOK_PER_PART)
    s = np.sort(ids, axis=1)
    return not np.any(s[:, 1:] == s[:, :-1])


def kernel(input, number_mask, n, concat, **_):
    assert int(n) == NSEG and int(concat) == 1
    if _rows_distinct(number_mask):
        nc = _get_nc()
        in_maps = make_in_maps(input, number_mask)
    else:
        nc = _get_nc_general()
        in_maps = make_in_maps_general(input, number_mask)
    res = bass_utils.run_bass_kernel_spmd(nc, in_maps, core_ids=list(range(NCORES)))
    return np.concatenate([res.results[c]["out"] for c in range(NCORES)], axis=0)
